# revision 27
# baseline (speedup 1.0000x reference)
"""MixHopConv (3 GIN hop-chains, N=50000, D=64, E=800000) on 8 TRN2 NeuronCores.

Self-contained Bass kernel: kernel(**inputs) takes the full (unsharded)
inputs and returns the full [50000, 64] float32 output.

v2 strategy (nodes sharded contiguously across 8 cores, 6250 each):
  - Round-1 source tables (padded x, [n,128] bf16 rows) are host-replicated
    parameters: no startup AllGather.
  - Per round: per-edge rows are fetched with dma_gather (SWDGE, 4 queues),
    aggregated by "staircase" PE matmuls against one-hot S matrices built on
    DVE from staged dst offsets. Gather issue runs SHIFT chunks ahead of
    consumption, A-half ahead of B-half, so the 4 descriptor-generation
    queues stay busy; the Pool instruction stream contains only gathers and
    the two AllGather issues per round (placed late to avoid head-of-line
    blocking at the in-order sequencer).
  - MLP algebra is fused: chains 2+3 run stacked on 128 partitions
    (block-diagonal weights); the output projection is folded into each
    chain's second MLP layer (W2@Wo precomputed), and all projection biases
    fold into a single bo'. The output accumulator lives in SBUF.
  - Round 1 ships [a|b] shards (transposed on PE) into bounce DRAM;
    AllGather A fires mid-round, B at round end.  Round 2 ships [junk|c].
    Round 3 consumes and writes the final output.
"""
import sys
import contextlib
import ctypes
import types

import numpy as np
import ml_dtypes

for _p in ("/opt/trn_rl_repo", "/opt/pypackages"):
    if _p not in sys.path:
        sys.path.append(_p)

import concourse.bass as bass
import concourse.bass_isa as bass_isa
import concourse.mybir as mybir
import concourse.tile as tile
import concourse.bacc as bacc
from concourse.bass import AP
from concourse.masks import make_identity
from concourse.library_config import mlp as mlp_lib

N_NODES = 50000
N_EDGES = 800000
N_CORES = 8
BF16 = mybir.dt.bfloat16
F32 = mybir.dt.float32
I16 = mybir.dt.int16

D = 64
CHUNK = 512
WIN = 128

SHIFT = 3      # B-half gather lags A-half by this many chunks
AG_POS_A = 8   # AllGather-A issued after this chunk's A-gather
AG_POS_B = 1   # prev round's AllGather-B issued after this chunk's A-gather
GMAX = 2048    # max idxs per dma_gather instruction (ring is 1024 descs/queue)


def make_config(n_nodes, n_edges, n_cores=8):
    assert n_nodes % n_cores == 0
    npc = n_nodes // n_cores
    chunks = []  # (cbase, [(sbase, w), ...])
    off = 0
    while off < npc:
        cw = min(CHUNK, npc - off)
        slots = []
        soff = 0
        while soff < cw:
            w = min(WIN, cw - soff)
            slots.append((soff, w))
            soff += w
        chunks.append((off, slots))
        off += cw
    # region A = first floor(nchunks/2) chunks per core: a slightly smaller A
    # lets AllGather-A fire earlier in the round (more wire overlap)
    ca = len(chunks) // 2
    splitA = sum(sum(w for (_, w) in sl) for (_, sl) in chunks[:ca])
    return dict(n_nodes=n_nodes, n_edges=n_edges, n_cores=n_cores, npc=npc,
                chunks=chunks, chunksA=ca, splitA=splitA)


def preprocess(cfg, edge_index):
    """Bucket/sort/pad edges; build per-core idx + dstrel arrays."""
    n_cores, npc = cfg["n_cores"], cfg["npc"]
    chunks = cfg["chunks"]
    splitA = cfg["splitA"]
    src = np.asarray(edge_index[0], dtype=np.int64)
    dst = np.asarray(edge_index[1], dtype=np.int64)

    buckets = [[[[None for _ in ch[1]] for _ in range(2)] for ch in chunks]
               for _ in range(n_cores)]
    core_of = dst // npc
    ldst = dst - core_of * npc
    cid = np.minimum(ldst // CHUNK, len(chunks) - 1)
    src_r = src // npc
    src_j = src - src_r * npc
    hid = (src_j >= splitA).astype(np.int64)
    splitB = npc - splitA
    tpos = np.where(hid == 0, src_r * splitA + src_j,
                    src_r * splitB + (src_j - splitA))
    for r in range(n_cores):
        m_r = core_of == r
        for c, (cbase, slots) in enumerate(chunks):
            m_rc = m_r & (cid == c)
            for s, (sbase, w) in enumerate(slots):
                m = m_rc & (ldst - cbase >= sbase) & (ldst - cbase < sbase + w)
                for h in range(2):
                    mh = m & (hid == h)
                    buckets[r][c][h][s] = (tpos[mh],
                                           (ldst[mh] - cbase - sbase))

    NB = []
    for c, (cbase, slots) in enumerate(chunks):
        NB_c = []
        for h in range(2):
            NB_ch = []
            for s in range(len(slots)):
                mx = max(len(buckets[r][c][h][s][0]) for r in range(n_cores))
                NB_ch.append(max(1, -(-mx // 128)))
            NB_c.append(NB_ch)
        NB.append(NB_c)
    NI = [[128 * sum(NB[c][h]) for h in range(2)] for c in range(len(chunks))]
    TOT = sum(NI[c][h] for c in range(len(chunks)) for h in range(2))
    NBTOT = sum(NB[c][h][s] for c in range(len(chunks)) for h in range(2)
                for s in range(len(chunks[c][1])))

    per_core = []
    for r in range(n_cores):
        idx_lin = np.zeros(TOT, dtype=np.int16)
        dr_lin = np.full((128, NBTOT), -1.0, dtype=np.float32)
        ioff = 0
        boff = 0
        for c in range(len(chunks)):
            for h in range(2):
                for s in range(len(chunks[c][1])):
                    srcs, drs = buckets[r][c][h][s]
                    nb = NB[c][h][s]
                    n = len(srcs)
                    idx_lin[ioff:ioff + n] = srcs.astype(np.int16)
                    for b in range(nb):
                        lo, hi = b * 128, min((b + 1) * 128, n)
                        if hi > lo:
                            dr_lin[0:hi - lo, boff + b] = drs[lo:hi]
                    ioff += nb * 128
                    boff += nb
        assert ioff == TOT and boff == NBTOT
        wrapped = idx_lin.reshape(TOT // 16, 16).T  # [16, TOT//16]
        idx_arr = np.tile(wrapped, (8, 1))
        per_core.append(dict(idx=np.ascontiguousarray(idx_arr),
                             dstrel=dr_lin.astype(ml_dtypes.bfloat16)))
    return dict(NB=NB, NI=NI, TOT=TOT, NBTOT=NBTOT, per_core=per_core)


def _bcast_mid(ap, n):
    """[P, W] AP -> [P, n, W] with middle dim broadcast."""
    return AP(ap.tensor, ap.offset, [ap.ap[0], [0, n], ap.ap[1]])


def relabel(cfg, edge_index):
    """Renumber nodes to balance in-edge load across (core, slot) buckets,
    shrinking the max-over-cores block padding. Returns (new_of_old,
    old_of_new) permutations."""
    n_nodes, n_cores, npc = cfg["n_nodes"], cfg["n_cores"], cfg["npc"]
    slotw = []
    for (cbase, slots) in cfg["chunks"]:
        slotw += [w for (_, w) in slots]
    nslot = len(slotw)
    slotw = np.asarray(slotw, np.int64)
    splitA_slots = sum(1 for i in range(nslot)
                       if sum(slotw[:i]) < cfg["splitA"])
    src = np.asarray(edge_index[0], np.int64)
    dst = np.asarray(edge_index[1], np.int64)
    deg = np.bincount(dst, minlength=n_nodes).astype(np.int64)
    nb = n_cores * nslot
    caps = np.tile(slotw, n_cores).astype(np.int64)

    # pass 1: LPT by total in-degree over all (core, slot) buckets
    load = np.zeros(nb)
    left = caps.copy()
    assign = np.empty(n_nodes, np.int32)
    for v in np.argsort(-deg, kind="stable"):
        b = int(np.argmin(np.where(left > 0, load, np.inf)))
        assign[v] = b
        load[b] += deg[v]
        left[b] -= 1
    # pass 2: regions now fixed; rebalance (A-count, B-count) within regions
    regA = (assign % nslot) < splitA_slots
    dA = np.bincount(dst[regA[src]], minlength=n_nodes).astype(np.int64)
    dB = deg - dA
    assign2 = np.empty(n_nodes, np.int32)
    for cls in (0, 1):
        inA = np.arange(nslot) < splitA_slots
        if cls == 0:
            nodes = np.where(regA)[0]
            buckets = np.where(np.tile(inA, n_cores))[0]
        else:
            nodes = np.where(~regA)[0]
            buckets = np.where(np.tile(~inA, n_cores))[0]
        bcaps = caps[buckets].copy()
        la = np.zeros(len(buckets))
        lb = np.zeros(len(buckets))
        tgtA = max(dA[nodes].sum() / len(buckets), 1.0)
        tgtB = max(dB[nodes].sum() / len(buckets), 1.0)
        for v in nodes[np.argsort(-(dA[nodes] + dB[nodes]), kind="stable")]:
            score = np.maximum((la + dA[v]) / tgtA, (lb + dB[v]) / tgtB)
            j = int(np.argmin(np.where(bcaps > 0, score, np.inf)))
            assign2[v] = buckets[j]
            la[j] += dA[v]
            lb[j] += dB[v]
            bcaps[j] -= 1

    new_of_old = np.empty(n_nodes, np.int64)
    slot_base = np.concatenate([[0], np.cumsum(slotw)])[:nslot]
    pos = np.zeros(nb, np.int64)
    for v in np.argsort(assign2, kind="stable"):
        b = assign2[v]
        core, slot = b // nslot, b % nslot
        new_of_old[v] = core * npc + slot_base[slot] + pos[b]
        pos[b] += 1
    old_of_new = np.empty(n_nodes, np.int64)
    old_of_new[new_of_old] = np.arange(n_nodes)
    return new_of_old, old_of_new


def build(cfg, pp):
    """Build the 8-core SPMD program. Returns nc."""
    n_nodes, n_cores, npc = cfg["n_nodes"], cfg["n_cores"], cfg["npc"]
    chunks = cfg["chunks"]
    ca, splitA = cfg["chunksA"], cfg["splitA"]
    splitB = npc - splitA
    nA, nB = n_cores * splitA, n_cores * splitB
    NB, NI, TOT, NBTOT = pp["NB"], pp["NI"], pp["TOT"], pp["NBTOT"]
    NCH = len(chunks)
    RG = [list(range(n_cores))]

    nc = bacc.Bacc("TRN2", target_bir_lowering=False, num_swdge_queues=4)

    # ---- parameters ----
    xtabA_in = nc.declare_dram_parameter("xtabA", [nA, 128], BF16, isOutput=False)
    xtabB_in = nc.declare_dram_parameter("xtabB", [nB, 128], BF16, isOutput=False)
    xt_in = nc.declare_dram_parameter("xt", [D, npc], BF16, isOutput=False)
    idx_in = nc.declare_dram_parameter("idx", [128, TOT // 16], I16, isOutput=False)
    dr_in = nc.declare_dram_parameter("dstrel", [128, NBTOT], BF16, isOutput=False)
    wdefs = {
        "l1_c1": ([D, D], BF16), "l2_c1": ([D, D], BF16),
        "l1_c23": ([D, 128], BF16), "l2_c23": ([128, 128], BF16),
        "l1_r2": ([128, 128], BF16), "l2_r2": ([128, 128], BF16),
        "l1_r3": ([D, D], BF16), "l2_r3": ([D, D], BF16),
        "b1_0": ([D, 1], F32), "b1_12": ([128, 1], F32),
        "b2_12": ([128, 1], F32), "b2_2": ([D, 1], F32),
        "b1_2": ([D, 1], F32), "bo": ([D, 1], F32),
    }
    w_in = {k: nc.declare_dram_parameter(k, shp, dt, isOutput=False)
            for k, (shp, dt) in wdefs.items()}
    out_ext = nc.declare_dram_parameter("out", [npc, D], F32, isOutput=True)

    # ---- internal DRAM ----
    bounceA = [nc.dram_tensor(f"bounceA{k}", [splitA, 128], BF16) for k in range(2)]
    bounceB = [nc.dram_tensor(f"bounceB{k}", [splitB, 128], BF16) for k in range(2)]
    tablesA = [nc.dram_tensor(f"tableA{k}", [nA, 128], BF16,
                              addr_space="Shared") for k in range(2)]
    tablesB = [nc.dram_tensor(f"tableB{k}", [nB, 128], BF16,
                              addr_space="Shared") for k in range(2)]

    with tile.TileContext(nc) as tc:
        nc.gpsimd.load_library(mlp_lib)
        with (
            tc.tile_pool(name="const", bufs=1) as constp,
            tc.tile_pool(name="resident", bufs=1) as resp,
            tc.tile_pool(name="gathA", bufs=SHIFT + 3) as gpoolA,
            tc.tile_pool(name="gathB", bufs=3) as gpoolB,
            tc.tile_pool(name="smat", bufs=2) as spool,
            tc.tile_pool(name="strip", bufs=2) as stripp,
            tc.tile_pool(name="psagg", bufs=2, space="PSUM") as psagg,
            tc.tile_pool(name="psmlp", bufs=2, space="PSUM") as psmlp,
            tc.tile_pool(name="pstp", bufs=1, space="PSUM") as pstp,
        ):
            # ---- constants / resident ----
            iota16 = constp.tile([128, WIN], I16)
            nc.gpsimd.iota(iota16[:], pattern=[[1, WIN]], base=0, channel_multiplier=0)
            iota_b = constp.tile([128, WIN], BF16)
            nc.vector.tensor_copy(iota_b[:], iota16[:])
            NBGMAX = max(NI[c][h] // 128 for c in range(NCH) for h in range(2))
            iota_rep = constp.tile([128, NBGMAX * WIN], BF16)
            rep = 1
            nc.vector.tensor_copy(iota_rep[:, 0:WIN], iota_b[:])
            while rep < NBGMAX:
                n = min(rep, NBGMAX - rep)
                nc.vector.tensor_copy(iota_rep[:, rep * WIN:(rep + n) * WIN],
                                      iota_rep[:, 0:n * WIN])
                rep += n
            id128 = constp.tile([128, 128], BF16)
            make_identity(nc, id128[:])
            id64 = constp.tile([D, D], F32)
            make_identity(nc, id64[:])

            # idx first: it gates the first gathers
            idx_t = resp.tile([128, TOT // 16], I16)
            nc.sync.dma_start(idx_t[:], idx_in[:])
            dr_t = resp.tile([128, NBTOT], BF16)
            nc.sync.dma_start(dr_t[:], dr_in[:])
            xt_t = resp.tile([D, npc], BF16, tag="xt", name="xt_t")
            nc.sync.dma_start(xt_t[:], xt_in[:])

            wt = {}
            for k, (shp, dt) in wdefs.items():
                wt[k] = constp.tile(shp, dt, tag=k, name=k)
                nc.sync.dma_start(wt[k][:], w_in[k][:])

            hn1 = resp.tile([128, npc], BF16, tag="hn1", name="hn1", bufs=1)
            hn2 = resp.tile([128, npc], BF16, tag="hn2", name="hn2", bufs=1)
            outacc = resp.tile([D, npc], F32, tag="oacc", name="outacc", bufs=1)
            # hn2's lower half ([junk|c] rows) is shipped but never consumed;
            # zero it once so the sim never sees uninitialized reads.
            nc.vector.memset(hn2[0:D, :], 0.0)

            # per-(chunk,half) idx offsets into idx_t / dr_t
            ioffs = {}
            boffs = {}
            io = bo_ = 0
            for c in range(NCH):
                for h in range(2):
                    ioffs[(c, h)] = io
                    boffs[(c, h)] = bo_
                    io += NI[c][h]
                    bo_ += NI[c][h] // 128
            assert io == TOT and bo_ == NBTOT

            gq = [0]

            def issue_gather(rk, c, h, tabA, tabB):
                ni = NI[c][h]
                nbg = ni // 128
                pool = gpoolA if h == 0 else gpoolB
                g = pool.tile([128, nbg, 128], BF16, tag=f"g{h}", name=f"g{h}_{rk}_{c}")
                src_ap = tabA[:] if h == 0 else tabB[:]
                io = ioffs[(c, h)]
                # split into <=GMAX-idx sub-gathers on rotating queues so each
                # fits the per-queue descriptor ring and drains in parallel
                off = 0
                while off < ni:
                    nsub = min(GMAX, ni - off)
                    b0, b1 = off // 128, (off + nsub) // 128
                    nc.gpsimd.dma_gather(
                        g[:, b0:b1, :], src_ap,
                        idx_t[:, (io + off) // 16:(io + off + nsub) // 16],
                        nsub, nsub, 128, elem_step=128, single_packet=False,
                        queue_num=gq[0] % 4)
                    gq[0] += 1
                    off += nsub
                return g

            def build_S(c, h):
                ni = NI[c][h]
                nbg = ni // 128
                slots = chunks[c][1]
                bo0 = boffs[(c, h)]
                S = spool.tile([128, nbg * WIN], BF16, tag=f"S{h}", name=f"S{h}_{c}")
                wmax = max(w for (_, w) in slots)
                if wmax == WIN:
                    nc.vector.tensor_tensor(
                        out=S[:],
                        in0=AP(dr_t[:].tensor,
                               dr_t[:, bo0:bo0 + nbg].offset,
                               [dr_t[:].ap[0], [1, nbg], [0, WIN]]),
                        in1=iota_rep[:, 0:nbg * WIN],
                        op=mybir.AluOpType.is_equal)
                else:
                    nc.vector.tensor_tensor(
                        out=AP(S[:].tensor, S[:].offset,
                               [S[:].ap[0], [WIN, nbg], [1, wmax]]),
                        in0=dr_t[:, bo0:bo0 + nbg].to_broadcast([128, nbg, wmax]),
                        in1=_bcast_mid(iota_b[:, :wmax], nbg),
                        op=mybir.AluOpType.is_equal)
                return S

            def staircase(c, gA, gB, SA, SB):
                """Accumulate this chunk's aggregation into a psum tile."""
                slots = chunks[c][1]
                ps = psagg.tile([128, CHUNK], F32, tag="agg", name=f"agg{c}")
                bcur = {0: 0, 1: 0}
                g_half = {0: gA, 1: gB}
                S_half = {0: SA, 1: SB}
                for s, (sbase, w) in enumerate(slots):
                    for h in range(2):
                        nb = NB[c][h][s]
                        for b in range(nb):
                            bi = bcur[h]
                            nc.tensor.matmul(
                                ps[:, sbase:sbase + w],
                                lhsT=g_half[h][:, bi, :],
                                rhs=S_half[h][:, bi * WIN:bi * WIN + w],
                                start=(h == 0 and b == 0),
                                stop=(h == 1 and b == nb - 1))
                            bcur[h] += 1
                return ps

            def ship(h_tile, c, bA, bB):
                """Transpose h_tile's chunk columns into bounce DRAM rows."""
                cbase, slots = chunks[c]
                cw = sum(w for (_, w) in slots)
                if cbase + cw <= splitA:
                    dstten, rowbase = bA, cbase
                else:
                    dstten, rowbase = bB, cbase - splitA
                t0 = 0
                while t0 < cw:
                    tw = min(128, cw - t0)
                    pt = pstp.tile([128, 128], BF16, tag="tp", name="tp")
                    nc.tensor.transpose(pt[:tw, :],
                                        h_tile[:, cbase + t0:cbase + t0 + tw],
                                        id128[:])
                    st = stripp.tile([128, 128], BF16, tag="shard", name="shard")
                    nc.vector.tensor_copy(st[:tw, :], pt[:tw, :])
                    nc.sync.dma_start(dstten[rowbase + t0:rowbase + t0 + tw, :],
                                      st[:tw, :])
                    t0 += tw

            def process(rk, c, gA, gB):
                cbase, slots = chunks[c]
                cw = sum(w for (_, w) in slots)
                SA = build_S(c, 0)
                SB = build_S(c, 1)
                ps = staircase(c, gA, gB, SA, SB)
                if rk == 1:
                    z1 = stripp.tile([D, CHUNK], BF16, tag="z1", name="z1")
                    nc.vector.tensor_tensor(z1[:, :cw], ps[0:D, :cw],
                                            xt_t[:, cbase:cbase + cw],
                                            op=mybir.AluOpType.add)
                    # chain 1 (hop-1): MLP0 with projection folded into layer 2
                    p1 = psmlp.tile([128, CHUNK], F32, tag="pm", name="p1")
                    nc.tensor.matmul(p1[0:D, :cw], lhsT=wt["l1_c1"][:],
                                     rhs=z1[:, :cw], start=True, stop=True)
                    m1 = stripp.tile([D, CHUNK], BF16, tag="m1", name="m1")
                    nc.scalar.activation(m1[:, :cw], p1[0:D, :cw],
                                         mybir.ActivationFunctionType.Relu,
                                         bias=wt["b1_0"][:])
                    po = psmlp.tile([128, CHUNK], F32, tag="po", name="po")
                    nc.tensor.matmul(po[0:D, :cw], lhsT=wt["l2_c1"][:],
                                     rhs=m1[:, :cw], start=True, stop=True)
                    nc.vector.tensor_copy(outacc[:, cbase:cbase + cw], po[0:D, :cw])
                    # chains 2+3 stacked: [a|b]
                    p1b = psmlp.tile([128, CHUNK], F32, tag="pm", name="p1b")
                    nc.tensor.matmul(p1b[:, :cw], lhsT=wt["l1_c23"][:],
                                     rhs=z1[:, :cw], start=True, stop=True)
                    mb = stripp.tile([128, CHUNK], BF16, tag="mb", name="mb")
                    nc.scalar.activation(mb[:, :cw], p1b[:, :cw],
                                         mybir.ActivationFunctionType.Relu,
                                         bias=wt["b1_12"][:])
                    p2b = psmlp.tile([128, CHUNK], F32, tag="po", name="p2b")
                    nc.tensor.matmul(p2b[:, :cw], lhsT=wt["l2_c23"][:],
                                     rhs=mb[:, :cw], start=True, stop=True)
                    nc.vector.tensor_tensor(hn1[:, cbase:cbase + cw],
                                            p2b[:, :cw],
                                            wt["b2_12"][:].to_broadcast([128, cw]),
                                            op=mybir.AluOpType.add)
                    ship(hn1, c, bounceA[0], bounceB[0])
                elif rk == 2:
                    zs = stripp.tile([128, CHUNK], BF16, tag="zs", name="zs")
                    nc.vector.tensor_tensor(zs[:, :cw], ps[:, :cw],
                                            hn1[:, cbase:cbase + cw],
                                            op=mybir.AluOpType.add)
                    p1 = psmlp.tile([128, CHUNK], F32, tag="pm", name="p1")
                    nc.tensor.matmul(p1[:, :cw], lhsT=wt["l1_r2"][:],
                                     rhs=zs[:, :cw], start=True, stop=True)
                    m = stripp.tile([128, CHUNK], BF16, tag="mb", name="m2")
                    nc.scalar.activation(m[:, :cw], p1[:, :cw],
                                         mybir.ActivationFunctionType.Relu,
                                         bias=wt["b1_12"][:])
                    p2 = psmlp.tile([128, CHUNK], F32, tag="po", name="p2")
                    nc.tensor.matmul(p2[:, :cw], lhsT=wt["l2_r2"][:],
                                     rhs=m[:, :cw], start=True, stop=True)
                    # p2 = [proj2 | c]
                    nc.vector.tensor_tensor(outacc[:, cbase:cbase + cw],
                                            outacc[:, cbase:cbase + cw],
                                            p2[0:D, :cw],
                                            op=mybir.AluOpType.add)
                    nc.vector.tensor_tensor(hn2[D:128, cbase:cbase + cw],
                                            p2[D:128, :cw],
                                            wt["b2_2"][:].to_broadcast([D, cw]),
                                            op=mybir.AluOpType.add)
                    ship(hn2, c, bounceA[1], bounceB[1])
                else:
                    z3 = stripp.tile([D, CHUNK], BF16, tag="z1", name="z3")
                    nc.vector.tensor_tensor(z3[:, :cw], ps[D:128, :cw],
                                            hn2[D:128, cbase:cbase + cw],
                                            op=mybir.AluOpType.add)
                    p1 = psmlp.tile([128, CHUNK], F32, tag="pm", name="p1")
                    nc.tensor.matmul(p1[0:D, :cw], lhsT=wt["l1_r3"][:],
                                     rhs=z3[:, :cw], start=True, stop=True)
                    m = stripp.tile([D, CHUNK], BF16, tag="m1", name="m3")
                    nc.scalar.activation(m[:, :cw], p1[0:D, :cw],
                                         mybir.ActivationFunctionType.Relu,
                                         bias=wt["b1_2"][:])
                    p2 = psmlp.tile([128, CHUNK], F32, tag="po", name="p2")
                    nc.tensor.matmul(p2[0:D, :cw], lhsT=wt["l2_r3"][:],
                                     rhs=m[:, :cw], start=True, stop=True)
                    fs = stripp.tile([D, CHUNK], F32, tag="fs", name="fs", bufs=1)
                    nc.scalar.activation(fs[:, :cw], p2[0:D, :cw],
                                         mybir.ActivationFunctionType.Identity,
                                         bias=wt["bo"][:])
                    nc.vector.tensor_tensor(fs[:, :cw], fs[:, :cw],
                                            outacc[:, cbase:cbase + cw],
                                            op=mybir.AluOpType.add)
                    t0 = 0
                    while t0 < cw:
                        tw = min(128, cw - t0)
                        pt = pstp.tile([128, D], F32, tag="ftp", name="ftp")
                        nc.tensor.transpose(pt[:tw, :], fs[:, t0:t0 + tw], id64[:])
                        os = stripp.tile([128, D], F32, tag="fout", name="fout")
                        nc.vector.tensor_copy(os[:tw, :], pt[:tw, :])
                        nc.sync.dma_start(out_ext[cbase + t0:cbase + t0 + tw, :],
                                          os[:tw, :])
                        t0 += tw

            def do_round(rk, tabA, tabB, tA_next, tB_next, bA, bB, prev_agB):
                gAs, gBs = {}, {}
                for i in range(NCH + SHIFT):
                    if i < NCH:
                        gAs[i] = issue_gather(rk, i, 0, tabA, tabB)
                    if i == AG_POS_B and prev_agB is not None:
                        # previous round's B-shard AllGather: issued here so it
                        # doesn't head-of-line-block this round's A gathers
                        pbB, ptB = prev_agB
                        nc.gpsimd.collective_compute(
                            "AllGather", mybir.AluOpType.bypass,
                            replica_groups=RG, ins=[pbB[:]], outs=[ptB[:]])
                    if i >= SHIFT:
                        c = i - SHIFT
                        gBs[c] = issue_gather(rk, c, 1, tabA, tabB)
                        process(rk, c, gAs.pop(c), gBs.pop(c))
                    # AG-A must be issued after process(ca-1)'s ship in program
                    # order (it reads those bounceA rows), hence after process.
                    if rk < 3 and i == AG_POS_A:
                        assert i - SHIFT >= ca - 1
                        nc.gpsimd.collective_compute(
                            "AllGather", mybir.AluOpType.bypass,
                            replica_groups=RG, ins=[bA[:]], outs=[tA_next[:]])

            do_round(1, xtabA_in, xtabB_in, tablesA[0], tablesB[0],
                     bounceA[0], bounceB[0], None)
            do_round(2, tablesA[0], tablesB[0], tablesA[1], tablesB[1],
                     bounceA[1], bounceB[1], (bounceB[0], tablesB[0]))
            do_round(3, tablesA[1], tablesB[1], None, None, None, None,
                     (bounceB[1], tablesB[1]))

    # Align each Pool-engine DMA's SWDGE queue with Tile's DMASW lane
    # rotation (lane = i % 8 over scheduled Pool DMA order; ucode requires a
    # lane's completion sem to be driven by a single queue).
    pool_dma_i = 0
    for f in nc.m.functions:
        for blk in f.blocks:
            for inst in blk.instructions:
                if (inst.engine == mybir.EngineType.Pool
                        and isinstance(inst, bass_isa.AnyDMAInstruction)
                        and not isinstance(inst, mybir.InstCollectiveCompute)):
                    if hasattr(inst, "queue_num"):
                        inst.queue_num = (pool_dma_i % 8) % 4
                    pool_dma_i += 1
    nc.compile()
    return nc


def host_inputs(cfg, pp, x, weights):
    """Build per-core in_maps. x: [n_nodes, 64] f32. weights: reference arrays."""
    n_cores, npc = cfg["n_cores"], cfg["npc"]
    splitA = cfg["splitA"]
    bf = ml_dtypes.bfloat16
    x = np.asarray(x, dtype=np.float32)

    W1 = [np.asarray(weights[f"W1_{i}"], np.float32) for i in range(3)]
    b1 = [np.asarray(weights[f"b1_{i}"], np.float32) for i in range(3)]
    W2 = [np.asarray(weights[f"W2_{i}"], np.float32) for i in range(3)]
    b2 = [np.asarray(weights[f"b2_{i}"], np.float32) for i in range(3)]
    Wo = np.asarray(weights["Wo"], np.float32).reshape(3, D, D)
    bo = np.asarray(weights["bo"], np.float32)

    W2o = [W2[i] @ Wo[i] for i in range(3)]
    bo_eff = bo + sum(Wo[i].T @ b2[i] for i in range(3))

    def bd(a, b_):
        out = np.zeros((128, 128), np.float32)
        out[0:D, 0:D] = a
        out[D:128, D:128] = b_
        return out

    wmats = {
        "l1_c1": W1[0].astype(bf),
        "l2_c1": W2o[0].astype(bf),
        "l1_c23": np.concatenate([W1[1], W1[2]], axis=1).astype(bf),
        "l2_c23": bd(W2[1], W2[2]).astype(bf),
        "l1_r2": bd(W1[1], W1[2]).astype(bf),
        "l2_r2": bd(W2o[1], W2[2]).astype(bf),
        "l1_r3": W1[2].astype(bf),
        "l2_r3": W2o[2].astype(bf),
        "b1_0": b1[0].reshape(D, 1),
        "b1_12": np.concatenate([b1[1], b1[2]]).reshape(128, 1),
        "b2_12": np.concatenate([b2[1], b2[2]]).reshape(128, 1),
        "b2_2": b2[2].reshape(D, 1),
        "b1_2": b1[2].reshape(D, 1),
        "bo": bo_eff.reshape(D, 1),
    }

    xpad = np.zeros((cfg["n_nodes"], 128), dtype=bf)
    xpad[:, :D] = x.astype(bf)
    xtabA = np.ascontiguousarray(np.concatenate(
        [xpad[r * npc:r * npc + splitA] for r in range(n_cores)]))
    xtabB = np.ascontiguousarray(np.concatenate(
        [xpad[r * npc + splitA:(r + 1) * npc] for r in range(n_cores)]))

    in_maps = []
    for r in range(n_cores):
        m = dict(wmats)
        m["xtabA"] = xtabA
        m["xtabB"] = xtabB
        xs = x[r * npc:(r + 1) * npc]
        m["xt"] = np.ascontiguousarray(xs.T.astype(bf))
        m["idx"] = pp["per_core"][r]["idx"]
        m["dstrel"] = pp["per_core"][r]["dstrel"]
        in_maps.append(m)
    return in_maps


_PROF_SO = "/opt/axon/libaxon_pjrt.so"


def _install_profile_shim():
    """Provide antenv.axon_hooks (absent in some containers) so
    run_bass_kernel_spmd(trace=True) can capture NTFF profiles."""
    try:
        import antenv
    except ImportError:
        return
    if getattr(antenv, "axon_hooks", None) is not None:
        return

    def _hook_factory(so_path):
        try:
            lib = ctypes.CDLL(so_path)
        except OSError:
            return None
        if not hasattr(lib, "axon_start_nrt_profile"):
            return None
        lib.axon_start_nrt_profile.argtypes = [ctypes.POINTER(ctypes.c_int64),
                                               ctypes.c_size_t]
        lib.axon_start_nrt_profile.restype = ctypes.c_int64
        lib.axon_stop_nrt_profile.argtypes = [ctypes.c_char_p]
        lib.axon_stop_nrt_profile.restype = ctypes.c_int64

        @contextlib.contextmanager
        def _hook(output_dir, device_ids):
            import jax
            jax.devices()
            if device_ids:
                ids = (ctypes.c_int64 * len(device_ids))(*device_ids)
                rc = lib.axon_start_nrt_profile(ids, len(device_ids))
            else:
                rc = lib.axon_start_nrt_profile(None, 0)
            if rc != 0:
                raise RuntimeError(f"axon_start_nrt_profile rc={rc}")
            try:
                yield
            finally:
                n = lib.axon_stop_nrt_profile(str(output_dir).encode())
                print(f"profile: {n} file(s) written to {output_dir}",
                      file=sys.stderr)

        return _hook

    mod = types.ModuleType("antenv.axon_hooks")
    _state = {"hook": _hook_factory(_PROF_SO)}
    mod.set_axon_ntff_profile_hook = lambda h: _state.__setitem__("hook", h)
    mod.get_axon_ntff_profile_hook = lambda: _state["hook"]
    sys.modules["antenv.axon_hooks"] = mod
    antenv.axon_hooks = mod
    import concourse.bass_utils as _bu
    _bu.upload_artifacts = lambda tmpdir: f"local://{tmpdir}"


_CACHE = {}


def _get_program(edge_index):
    key = hash(edge_index.tobytes())
    if key not in _CACHE:
        cfg = make_config(N_NODES, N_EDGES, N_CORES)
        new_of_old, old_of_new = relabel(cfg, edge_index)
        ei2 = np.stack([new_of_old[edge_index[0]], new_of_old[edge_index[1]]])
        pp = preprocess(cfg, ei2)
        nc = build(cfg, pp)
        _CACHE[key] = (cfg, pp, nc, new_of_old, old_of_new)
    return _CACHE[key]


def run(trace=False, **inputs):
    """Run the kernel; returns (output [N_NODES, 64] f32, exec_time_ns|None)."""
    from concourse.bass_utils import run_bass_kernel_spmd

    x = np.asarray(inputs["x"], dtype=np.float32)
    edge_index = np.asarray(inputs["edge_index"], dtype=np.int64)
    weights = {k: np.asarray(v) for k, v in inputs.items()
               if k not in ("x", "edge_index")}
    assert x.shape == (N_NODES, D) and edge_index.shape == (2, N_EDGES)

    if trace:
        _install_profile_shim()
    cfg, pp, nc, new_of_old, old_of_new = _get_program(edge_index)
    in_maps = host_inputs(cfg, pp, x[old_of_new], weights)
    res = run_bass_kernel_spmd(nc, in_maps, list(range(N_CORES)), trace=trace)
    out = np.concatenate([res.results[r]["out"] for r in range(N_CORES)],
                         axis=0).astype(np.float32)
    return out[new_of_old], res.exec_time_ns


def kernel(**inputs):
    out, _ = run(trace=False, **inputs)
    return out


# revision 30
# speedup vs baseline: 1.0037x; 1.0037x over previous
"""MixHopConv (3 GIN hop-chains, N=50000, D=64, E=800000) on 8 TRN2 NeuronCores.

Self-contained Bass kernel: kernel(**inputs) takes the full (unsharded)
inputs and returns the full [50000, 64] float32 output.

v2 strategy (nodes sharded contiguously across 8 cores, 6250 each):
  - Round-1 source tables (padded x, [n,128] bf16 rows) are host-replicated
    parameters: no startup AllGather.
  - Per round: per-edge rows are fetched with dma_gather (SWDGE, 4 queues),
    aggregated by "staircase" PE matmuls against one-hot S matrices built on
    DVE from staged dst offsets. Gather issue runs SHIFT chunks ahead of
    consumption, A-half ahead of B-half, so the 4 descriptor-generation
    queues stay busy; the Pool instruction stream contains only gathers and
    the two AllGather issues per round (placed late to avoid head-of-line
    blocking at the in-order sequencer).
  - MLP algebra is fused: chains 2+3 run stacked on 128 partitions
    (block-diagonal weights); the output projection is folded into each
    chain's second MLP layer (W2@Wo precomputed), and all projection biases
    fold into a single bo'. The output accumulator lives in SBUF.
  - Round 1 ships [a|b] shards (transposed on PE) into bounce DRAM;
    AllGather A fires mid-round, B at round end.  Round 2 ships [junk|c].
    Round 3 consumes and writes the final output.
"""
import sys
import contextlib
import ctypes
import types

import numpy as np
import ml_dtypes

for _p in ("/opt/trn_rl_repo", "/opt/pypackages"):
    if _p not in sys.path:
        sys.path.append(_p)

import concourse.bass as bass
import concourse.bass_isa as bass_isa
import concourse.mybir as mybir
import concourse.tile as tile
import concourse.bacc as bacc
from concourse.bass import AP
from concourse.masks import make_identity
from concourse.library_config import mlp as mlp_lib

N_NODES = 50000
N_EDGES = 800000
N_CORES = 8
BF16 = mybir.dt.bfloat16
F32 = mybir.dt.float32
I16 = mybir.dt.int16

D = 64
CHUNK = 512
WIN = 128

SHIFT = 3      # B-half gather lags A-half by this many chunks
AG_POS_A = 8   # AllGather-A issued after this chunk's A-gather
AG_POS_B = 1   # prev round's AllGather-B issued after this chunk's A-gather
GMAX = 1024    # max idxs per dma_gather instruction


def make_config(n_nodes, n_edges, n_cores=8):
    assert n_nodes % n_cores == 0
    npc = n_nodes // n_cores
    chunks = []  # (cbase, [(sbase, w), ...])
    off = 0
    while off < npc:
        cw = min(CHUNK, npc - off)
        slots = []
        soff = 0
        while soff < cw:
            w = min(WIN, cw - soff)
            slots.append((soff, w))
            soff += w
        chunks.append((off, slots))
        off += cw
    # region A = first floor(nchunks/2) chunks per core: a slightly smaller A
    # lets AllGather-A fire earlier in the round (more wire overlap)
    ca = len(chunks) // 2
    splitA = sum(sum(w for (_, w) in sl) for (_, sl) in chunks[:ca])
    return dict(n_nodes=n_nodes, n_edges=n_edges, n_cores=n_cores, npc=npc,
                chunks=chunks, chunksA=ca, splitA=splitA)


def preprocess(cfg, edge_index):
    """Bucket/sort/pad edges; build per-core idx + dstrel arrays."""
    n_cores, npc = cfg["n_cores"], cfg["npc"]
    chunks = cfg["chunks"]
    splitA = cfg["splitA"]
    src = np.asarray(edge_index[0], dtype=np.int64)
    dst = np.asarray(edge_index[1], dtype=np.int64)

    buckets = [[[[None for _ in ch[1]] for _ in range(2)] for ch in chunks]
               for _ in range(n_cores)]
    core_of = dst // npc
    ldst = dst - core_of * npc
    cid = np.minimum(ldst // CHUNK, len(chunks) - 1)
    src_r = src // npc
    src_j = src - src_r * npc
    hid = (src_j >= splitA).astype(np.int64)
    splitB = npc - splitA
    tpos = np.where(hid == 0, src_r * splitA + src_j,
                    src_r * splitB + (src_j - splitA))
    for r in range(n_cores):
        m_r = core_of == r
        for c, (cbase, slots) in enumerate(chunks):
            m_rc = m_r & (cid == c)
            for s, (sbase, w) in enumerate(slots):
                m = m_rc & (ldst - cbase >= sbase) & (ldst - cbase < sbase + w)
                for h in range(2):
                    mh = m & (hid == h)
                    buckets[r][c][h][s] = (tpos[mh],
                                           (ldst[mh] - cbase - sbase))

    NB = []
    for c, (cbase, slots) in enumerate(chunks):
        NB_c = []
        for h in range(2):
            NB_ch = []
            for s in range(len(slots)):
                mx = max(len(buckets[r][c][h][s][0]) for r in range(n_cores))
                NB_ch.append(max(1, -(-mx // 128)))
            NB_c.append(NB_ch)
        NB.append(NB_c)
    NI = [[128 * sum(NB[c][h]) for h in range(2)] for c in range(len(chunks))]
    TOT = sum(NI[c][h] for c in range(len(chunks)) for h in range(2))
    NBTOT = sum(NB[c][h][s] for c in range(len(chunks)) for h in range(2)
                for s in range(len(chunks[c][1])))

    per_core = []
    for r in range(n_cores):
        idx_lin = np.zeros(TOT, dtype=np.int16)
        dr_lin = np.full((128, NBTOT), -1.0, dtype=np.float32)
        ioff = 0
        boff = 0
        for c in range(len(chunks)):
            for h in range(2):
                for s in range(len(chunks[c][1])):
                    srcs, drs = buckets[r][c][h][s]
                    nb = NB[c][h][s]
                    n = len(srcs)
                    idx_lin[ioff:ioff + n] = srcs.astype(np.int16)
                    for b in range(nb):
                        lo, hi = b * 128, min((b + 1) * 128, n)
                        if hi > lo:
                            dr_lin[0:hi - lo, boff + b] = drs[lo:hi]
                    ioff += nb * 128
                    boff += nb
        assert ioff == TOT and boff == NBTOT
        wrapped = idx_lin.reshape(TOT // 16, 16).T  # [16, TOT//16]
        idx_arr = np.tile(wrapped, (8, 1))
        per_core.append(dict(idx=np.ascontiguousarray(idx_arr),
                             dstrel=dr_lin.astype(ml_dtypes.bfloat16)))
    return dict(NB=NB, NI=NI, TOT=TOT, NBTOT=NBTOT, per_core=per_core)


def _bcast_mid(ap, n):
    """[P, W] AP -> [P, n, W] with middle dim broadcast."""
    return AP(ap.tensor, ap.offset, [ap.ap[0], [0, n], ap.ap[1]])


def relabel(cfg, edge_index):
    """Renumber nodes to balance in-edge load across (core, slot) buckets,
    shrinking the max-over-cores block padding. Returns (new_of_old,
    old_of_new) permutations."""
    n_nodes, n_cores, npc = cfg["n_nodes"], cfg["n_cores"], cfg["npc"]
    slotw = []
    for (cbase, slots) in cfg["chunks"]:
        slotw += [w for (_, w) in slots]
    nslot = len(slotw)
    slotw = np.asarray(slotw, np.int64)
    splitA_slots = sum(1 for i in range(nslot)
                       if sum(slotw[:i]) < cfg["splitA"])
    src = np.asarray(edge_index[0], np.int64)
    dst = np.asarray(edge_index[1], np.int64)
    deg = np.bincount(dst, minlength=n_nodes).astype(np.int64)
    nb = n_cores * nslot
    caps = np.tile(slotw, n_cores).astype(np.int64)

    # pass 1: LPT by total in-degree over all (core, slot) buckets
    load = np.zeros(nb)
    left = caps.copy()
    assign = np.empty(n_nodes, np.int32)
    for v in np.argsort(-deg, kind="stable"):
        b = int(np.argmin(np.where(left > 0, load, np.inf)))
        assign[v] = b
        load[b] += deg[v]
        left[b] -= 1
    # pass 2: regions now fixed; rebalance (A-count, B-count) within regions
    regA = (assign % nslot) < splitA_slots
    dA = np.bincount(dst[regA[src]], minlength=n_nodes).astype(np.int64)
    dB = deg - dA
    assign2 = np.empty(n_nodes, np.int32)
    for cls in (0, 1):
        inA = np.arange(nslot) < splitA_slots
        if cls == 0:
            nodes = np.where(regA)[0]
            buckets = np.where(np.tile(inA, n_cores))[0]
        else:
            nodes = np.where(~regA)[0]
            buckets = np.where(np.tile(~inA, n_cores))[0]
        bcaps = caps[buckets].copy()
        la = np.zeros(len(buckets))
        lb = np.zeros(len(buckets))
        tgtA = max(dA[nodes].sum() / len(buckets), 1.0)
        tgtB = max(dB[nodes].sum() / len(buckets), 1.0)
        for v in nodes[np.argsort(-(dA[nodes] + dB[nodes]), kind="stable")]:
            score = np.maximum((la + dA[v]) / tgtA, (lb + dB[v]) / tgtB)
            j = int(np.argmin(np.where(bcaps > 0, score, np.inf)))
            assign2[v] = buckets[j]
            la[j] += dA[v]
            lb[j] += dB[v]
            bcaps[j] -= 1

    new_of_old = np.empty(n_nodes, np.int64)
    slot_base = np.concatenate([[0], np.cumsum(slotw)])[:nslot]
    pos = np.zeros(nb, np.int64)
    for v in np.argsort(assign2, kind="stable"):
        b = assign2[v]
        core, slot = b // nslot, b % nslot
        new_of_old[v] = core * npc + slot_base[slot] + pos[b]
        pos[b] += 1
    old_of_new = np.empty(n_nodes, np.int64)
    old_of_new[new_of_old] = np.arange(n_nodes)
    return new_of_old, old_of_new


def build(cfg, pp):
    """Build the 8-core SPMD program. Returns nc."""
    n_nodes, n_cores, npc = cfg["n_nodes"], cfg["n_cores"], cfg["npc"]
    chunks = cfg["chunks"]
    ca, splitA = cfg["chunksA"], cfg["splitA"]
    splitB = npc - splitA
    nA, nB = n_cores * splitA, n_cores * splitB
    NB, NI, TOT, NBTOT = pp["NB"], pp["NI"], pp["TOT"], pp["NBTOT"]
    NCH = len(chunks)
    RG = [list(range(n_cores))]

    nc = bacc.Bacc("TRN2", target_bir_lowering=False, num_swdge_queues=4)

    # ---- parameters ----
    xtabA_in = nc.declare_dram_parameter("xtabA", [nA, 128], BF16, isOutput=False)
    xtabB_in = nc.declare_dram_parameter("xtabB", [nB, 128], BF16, isOutput=False)
    xt_in = nc.declare_dram_parameter("xt", [D, npc], BF16, isOutput=False)
    idx_in = nc.declare_dram_parameter("idx", [128, TOT // 16], I16, isOutput=False)
    dr_in = nc.declare_dram_parameter("dstrel", [128, NBTOT], BF16, isOutput=False)
    wdefs = {
        "l1_c1": ([D, D], BF16), "l2_c1": ([D, D], BF16),
        "l1_c23": ([D, 128], BF16), "l2_c23": ([128, 128], BF16),
        "l1_r2": ([128, 128], BF16), "l2_r2": ([128, 128], BF16),
        "l1_r3": ([D, D], BF16), "l2_r3": ([D, D], BF16),
        "b1_0": ([D, 1], F32), "b1_12": ([128, 1], F32),
        "b2_12": ([128, 1], F32), "b2_2": ([D, 1], F32),
        "b1_2": ([D, 1], F32), "bo": ([D, 1], F32),
    }
    w_in = {k: nc.declare_dram_parameter(k, shp, dt, isOutput=False)
            for k, (shp, dt) in wdefs.items()}
    out_ext = nc.declare_dram_parameter("out", [npc, D], F32, isOutput=True)

    # ---- internal DRAM ----
    bounceA = [nc.dram_tensor(f"bounceA{k}", [splitA, 128], BF16) for k in range(2)]
    bounceB = [nc.dram_tensor(f"bounceB{k}", [splitB, 128], BF16) for k in range(2)]
    tablesA = [nc.dram_tensor(f"tableA{k}", [nA, 128], BF16,
                              addr_space="Shared") for k in range(2)]
    tablesB = [nc.dram_tensor(f"tableB{k}", [nB, 128], BF16,
                              addr_space="Shared") for k in range(2)]

    with tile.TileContext(nc) as tc:
        nc.gpsimd.load_library(mlp_lib)
        with (
            tc.tile_pool(name="const", bufs=1) as constp,
            tc.tile_pool(name="resident", bufs=1) as resp,
            tc.tile_pool(name="gathA", bufs=SHIFT + 3) as gpoolA,
            tc.tile_pool(name="gathB", bufs=2) as gpoolB,
            tc.tile_pool(name="smat", bufs=2) as spool,
            tc.tile_pool(name="strip", bufs=2) as stripp,
            tc.tile_pool(name="psagg", bufs=2, space="PSUM") as psagg,
            tc.tile_pool(name="psmlp", bufs=2, space="PSUM") as psmlp,
            tc.tile_pool(name="pstp", bufs=1, space="PSUM") as pstp,
        ):
            # ---- constants / resident ----
            iota16 = constp.tile([128, WIN], I16)
            nc.gpsimd.iota(iota16[:], pattern=[[1, WIN]], base=0, channel_multiplier=0)
            iota_b = constp.tile([128, WIN], BF16)
            nc.vector.tensor_copy(iota_b[:], iota16[:])
            NBGMAX = max(NI[c][h] // 128 for c in range(NCH) for h in range(2))
            iota_rep = constp.tile([128, NBGMAX * WIN], BF16)
            rep = 1
            nc.vector.tensor_copy(iota_rep[:, 0:WIN], iota_b[:])
            while rep < NBGMAX:
                n = min(rep, NBGMAX - rep)
                nc.vector.tensor_copy(iota_rep[:, rep * WIN:(rep + n) * WIN],
                                      iota_rep[:, 0:n * WIN])
                rep += n
            id128 = constp.tile([128, 128], BF16)
            make_identity(nc, id128[:])
            id64 = constp.tile([D, D], F32)
            make_identity(nc, id64[:])

            # idx first: it gates the first gathers
            idx_t = resp.tile([128, TOT // 16], I16)
            nc.sync.dma_start(idx_t[:], idx_in[:])
            dr_t = resp.tile([128, NBTOT], BF16)
            nc.sync.dma_start(dr_t[:], dr_in[:])
            xt_t = resp.tile([D, npc], BF16, tag="xt", name="xt_t")
            nc.sync.dma_start(xt_t[:], xt_in[:])

            wt = {}
            for k, (shp, dt) in wdefs.items():
                wt[k] = constp.tile(shp, dt, tag=k, name=k)
                nc.sync.dma_start(wt[k][:], w_in[k][:])

            hn1 = resp.tile([128, npc], BF16, tag="hn1", name="hn1", bufs=1)
            hn2 = resp.tile([128, npc], BF16, tag="hn2", name="hn2", bufs=1)
            outacc = resp.tile([D, npc], F32, tag="oacc", name="outacc", bufs=1)
            # hn2's lower half ([junk|c] rows) is shipped but never consumed;
            # zero it once so the sim never sees uninitialized reads.
            nc.vector.memset(hn2[0:D, :], 0.0)

            # per-(chunk,half) idx offsets into idx_t / dr_t
            ioffs = {}
            boffs = {}
            io = bo_ = 0
            for c in range(NCH):
                for h in range(2):
                    ioffs[(c, h)] = io
                    boffs[(c, h)] = bo_
                    io += NI[c][h]
                    bo_ += NI[c][h] // 128
            assert io == TOT and bo_ == NBTOT

            gq = [0]

            def issue_gather(rk, c, h, tabA, tabB):
                ni = NI[c][h]
                nbg = ni // 128
                pool = gpoolA if h == 0 else gpoolB
                g = pool.tile([128, nbg, 128], BF16, tag=f"g{h}", name=f"g{h}_{rk}_{c}")
                src_ap = tabA[:] if h == 0 else tabB[:]
                io = ioffs[(c, h)]
                # split into <=GMAX-idx sub-gathers on rotating queues so each
                # fits the per-queue descriptor ring and drains in parallel
                off = 0
                while off < ni:
                    nsub = min(GMAX, ni - off)
                    b0, b1 = off // 128, (off + nsub) // 128
                    nc.gpsimd.dma_gather(
                        g[:, b0:b1, :], src_ap,
                        idx_t[:, (io + off) // 16:(io + off + nsub) // 16],
                        nsub, nsub, 128, elem_step=128, single_packet=False,
                        queue_num=gq[0] % 4)
                    gq[0] += 1
                    off += nsub
                return g

            def build_S(c, h):
                ni = NI[c][h]
                nbg = ni // 128
                slots = chunks[c][1]
                bo0 = boffs[(c, h)]
                S = spool.tile([128, nbg * WIN], BF16, tag=f"S{h}", name=f"S{h}_{c}")
                wmax = max(w for (_, w) in slots)
                if wmax == WIN:
                    nc.vector.tensor_tensor(
                        out=S[:],
                        in0=AP(dr_t[:].tensor,
                               dr_t[:, bo0:bo0 + nbg].offset,
                               [dr_t[:].ap[0], [1, nbg], [0, WIN]]),
                        in1=iota_rep[:, 0:nbg * WIN],
                        op=mybir.AluOpType.is_equal)
                else:
                    nc.vector.tensor_tensor(
                        out=AP(S[:].tensor, S[:].offset,
                               [S[:].ap[0], [WIN, nbg], [1, wmax]]),
                        in0=dr_t[:, bo0:bo0 + nbg].to_broadcast([128, nbg, wmax]),
                        in1=_bcast_mid(iota_b[:, :wmax], nbg),
                        op=mybir.AluOpType.is_equal)
                return S

            def staircase(c, gA, gB, SA, SB):
                """Accumulate this chunk's aggregation into a psum tile."""
                slots = chunks[c][1]
                ps = psagg.tile([128, CHUNK], F32, tag="agg", name=f"agg{c}")
                bcur = {0: 0, 1: 0}
                g_half = {0: gA, 1: gB}
                S_half = {0: SA, 1: SB}
                for s, (sbase, w) in enumerate(slots):
                    for h in range(2):
                        nb = NB[c][h][s]
                        for b in range(nb):
                            bi = bcur[h]
                            nc.tensor.matmul(
                                ps[:, sbase:sbase + w],
                                lhsT=g_half[h][:, bi, :],
                                rhs=S_half[h][:, bi * WIN:bi * WIN + w],
                                start=(h == 0 and b == 0),
                                stop=(h == 1 and b == nb - 1))
                            bcur[h] += 1
                return ps

            def ship(h_tile, c, bA, bB):
                """Transpose h_tile's chunk columns into bounce DRAM rows."""
                cbase, slots = chunks[c]
                cw = sum(w for (_, w) in slots)
                if cbase + cw <= splitA:
                    dstten, rowbase = bA, cbase
                else:
                    dstten, rowbase = bB, cbase - splitA
                t0 = 0
                while t0 < cw:
                    tw = min(128, cw - t0)
                    pt = pstp.tile([128, 128], BF16, tag="tp", name="tp")
                    nc.tensor.transpose(pt[:tw, :],
                                        h_tile[:, cbase + t0:cbase + t0 + tw],
                                        id128[:])
                    st = stripp.tile([128, 128], BF16, tag="shard", name="shard")
                    nc.vector.tensor_copy(st[:tw, :], pt[:tw, :])
                    nc.sync.dma_start(dstten[rowbase + t0:rowbase + t0 + tw, :],
                                      st[:tw, :])
                    t0 += tw

            def process(rk, c, gA, gB):
                cbase, slots = chunks[c]
                cw = sum(w for (_, w) in slots)
                SA = build_S(c, 0)
                SB = build_S(c, 1)
                ps = staircase(c, gA, gB, SA, SB)
                if rk == 1:
                    z1 = stripp.tile([D, CHUNK], BF16, tag="z1", name="z1")
                    nc.vector.tensor_tensor(z1[:, :cw], ps[0:D, :cw],
                                            xt_t[:, cbase:cbase + cw],
                                            op=mybir.AluOpType.add)
                    # chain 1 (hop-1): MLP0 with projection folded into layer 2
                    p1 = psmlp.tile([128, CHUNK], F32, tag="pm", name="p1")
                    nc.tensor.matmul(p1[0:D, :cw], lhsT=wt["l1_c1"][:],
                                     rhs=z1[:, :cw], start=True, stop=True)
                    m1 = stripp.tile([D, CHUNK], BF16, tag="m1", name="m1")
                    nc.scalar.activation(m1[:, :cw], p1[0:D, :cw],
                                         mybir.ActivationFunctionType.Relu,
                                         bias=wt["b1_0"][:])
                    po = psmlp.tile([128, CHUNK], F32, tag="po", name="po")
                    nc.tensor.matmul(po[0:D, :cw], lhsT=wt["l2_c1"][:],
                                     rhs=m1[:, :cw], start=True, stop=True)
                    nc.vector.tensor_copy(outacc[:, cbase:cbase + cw], po[0:D, :cw])
                    # chains 2+3 stacked: [a|b]
                    p1b = psmlp.tile([128, CHUNK], F32, tag="pm", name="p1b")
                    nc.tensor.matmul(p1b[:, :cw], lhsT=wt["l1_c23"][:],
                                     rhs=z1[:, :cw], start=True, stop=True)
                    mb = stripp.tile([128, CHUNK], BF16, tag="mb", name="mb")
                    nc.scalar.activation(mb[:, :cw], p1b[:, :cw],
                                         mybir.ActivationFunctionType.Relu,
                                         bias=wt["b1_12"][:])
                    p2b = psmlp.tile([128, CHUNK], F32, tag="po", name="p2b")
                    nc.tensor.matmul(p2b[:, :cw], lhsT=wt["l2_c23"][:],
                                     rhs=mb[:, :cw], start=True, stop=True)
                    nc.vector.tensor_tensor(hn1[:, cbase:cbase + cw],
                                            p2b[:, :cw],
                                            wt["b2_12"][:].to_broadcast([128, cw]),
                                            op=mybir.AluOpType.add)
                    ship(hn1, c, bounceA[0], bounceB[0])
                elif rk == 2:
                    zs = stripp.tile([128, CHUNK], BF16, tag="zs", name="zs")
                    nc.vector.tensor_tensor(zs[:, :cw], ps[:, :cw],
                                            hn1[:, cbase:cbase + cw],
                                            op=mybir.AluOpType.add)
                    p1 = psmlp.tile([128, CHUNK], F32, tag="pm", name="p1")
                    nc.tensor.matmul(p1[:, :cw], lhsT=wt["l1_r2"][:],
                                     rhs=zs[:, :cw], start=True, stop=True)
                    m = stripp.tile([128, CHUNK], BF16, tag="mb", name="m2")
                    nc.scalar.activation(m[:, :cw], p1[:, :cw],
                                         mybir.ActivationFunctionType.Relu,
                                         bias=wt["b1_12"][:])
                    p2 = psmlp.tile([128, CHUNK], F32, tag="po", name="p2")
                    nc.tensor.matmul(p2[:, :cw], lhsT=wt["l2_r2"][:],
                                     rhs=m[:, :cw], start=True, stop=True)
                    # p2 = [proj2 | c]
                    nc.vector.tensor_tensor(outacc[:, cbase:cbase + cw],
                                            outacc[:, cbase:cbase + cw],
                                            p2[0:D, :cw],
                                            op=mybir.AluOpType.add)
                    nc.vector.tensor_tensor(hn2[D:128, cbase:cbase + cw],
                                            p2[D:128, :cw],
                                            wt["b2_2"][:].to_broadcast([D, cw]),
                                            op=mybir.AluOpType.add)
                    ship(hn2, c, bounceA[1], bounceB[1])
                else:
                    z3 = stripp.tile([D, CHUNK], BF16, tag="z1", name="z3")
                    nc.vector.tensor_tensor(z3[:, :cw], ps[D:128, :cw],
                                            hn2[D:128, cbase:cbase + cw],
                                            op=mybir.AluOpType.add)
                    p1 = psmlp.tile([128, CHUNK], F32, tag="pm", name="p1")
                    nc.tensor.matmul(p1[0:D, :cw], lhsT=wt["l1_r3"][:],
                                     rhs=z3[:, :cw], start=True, stop=True)
                    m = stripp.tile([D, CHUNK], BF16, tag="m1", name="m3")
                    nc.scalar.activation(m[:, :cw], p1[0:D, :cw],
                                         mybir.ActivationFunctionType.Relu,
                                         bias=wt["b1_2"][:])
                    p2 = psmlp.tile([128, CHUNK], F32, tag="po", name="p2")
                    nc.tensor.matmul(p2[0:D, :cw], lhsT=wt["l2_r3"][:],
                                     rhs=m[:, :cw], start=True, stop=True)
                    fs = stripp.tile([D, CHUNK], F32, tag="fs", name="fs")
                    nc.scalar.activation(fs[:, :cw], p2[0:D, :cw],
                                         mybir.ActivationFunctionType.Identity,
                                         bias=wt["bo"][:])
                    nc.vector.tensor_tensor(fs[:, :cw], fs[:, :cw],
                                            outacc[:, cbase:cbase + cw],
                                            op=mybir.AluOpType.add)
                    t0 = 0
                    while t0 < cw:
                        tw = min(128, cw - t0)
                        pt = pstp.tile([128, D], F32, tag="ftp", name="ftp")
                        nc.tensor.transpose(pt[:tw, :], fs[:, t0:t0 + tw], id64[:])
                        os = stripp.tile([128, D], F32, tag="fout", name="fout")
                        nc.vector.tensor_copy(os[:tw, :], pt[:tw, :])
                        nc.sync.dma_start(out_ext[cbase + t0:cbase + t0 + tw, :],
                                          os[:tw, :])
                        t0 += tw

            def do_round(rk, tabA, tabB, tA_next, tB_next, bA, bB, prev_agB):
                gAs, gBs = {}, {}
                for i in range(NCH + SHIFT):
                    if i < NCH:
                        gAs[i] = issue_gather(rk, i, 0, tabA, tabB)
                    if i == AG_POS_B and prev_agB is not None:
                        # previous round's B-shard AllGather: issued here so it
                        # doesn't head-of-line-block this round's A gathers
                        pbB, ptB = prev_agB
                        nc.gpsimd.collective_compute(
                            "AllGather", mybir.AluOpType.bypass,
                            replica_groups=RG, ins=[pbB[:]], outs=[ptB[:]])
                    if i >= SHIFT:
                        c = i - SHIFT
                        gBs[c] = issue_gather(rk, c, 1, tabA, tabB)
                        process(rk, c, gAs.pop(c), gBs.pop(c))
                    # AG-A must be issued after process(ca-1)'s ship in program
                    # order (it reads those bounceA rows), hence after process.
                    if rk < 3 and i == AG_POS_A:
                        assert i - SHIFT >= ca - 1
                        nc.gpsimd.collective_compute(
                            "AllGather", mybir.AluOpType.bypass,
                            replica_groups=RG, ins=[bA[:]], outs=[tA_next[:]])

            do_round(1, xtabA_in, xtabB_in, tablesA[0], tablesB[0],
                     bounceA[0], bounceB[0], None)
            do_round(2, tablesA[0], tablesB[0], tablesA[1], tablesB[1],
                     bounceA[1], bounceB[1], (bounceB[0], tablesB[0]))
            do_round(3, tablesA[1], tablesB[1], None, None, None, None,
                     (bounceB[1], tablesB[1]))

    # Align each Pool-engine DMA's SWDGE queue with Tile's DMASW lane
    # rotation (lane = i % 8 over scheduled Pool DMA order; ucode requires a
    # lane's completion sem to be driven by a single queue).
    pool_dma_i = 0
    for f in nc.m.functions:
        for blk in f.blocks:
            for inst in blk.instructions:
                if (inst.engine == mybir.EngineType.Pool
                        and isinstance(inst, bass_isa.AnyDMAInstruction)
                        and not isinstance(inst, mybir.InstCollectiveCompute)):
                    if hasattr(inst, "queue_num"):
                        inst.queue_num = (pool_dma_i % 8) % 4
                    pool_dma_i += 1
    nc.compile()
    return nc


def host_inputs(cfg, pp, x, weights):
    """Build per-core in_maps. x: [n_nodes, 64] f32. weights: reference arrays."""
    n_cores, npc = cfg["n_cores"], cfg["npc"]
    splitA = cfg["splitA"]
    bf = ml_dtypes.bfloat16
    x = np.asarray(x, dtype=np.float32)

    W1 = [np.asarray(weights[f"W1_{i}"], np.float32) for i in range(3)]
    b1 = [np.asarray(weights[f"b1_{i}"], np.float32) for i in range(3)]
    W2 = [np.asarray(weights[f"W2_{i}"], np.float32) for i in range(3)]
    b2 = [np.asarray(weights[f"b2_{i}"], np.float32) for i in range(3)]
    Wo = np.asarray(weights["Wo"], np.float32).reshape(3, D, D)
    bo = np.asarray(weights["bo"], np.float32)

    W2o = [W2[i] @ Wo[i] for i in range(3)]
    bo_eff = bo + sum(Wo[i].T @ b2[i] for i in range(3))

    def bd(a, b_):
        out = np.zeros((128, 128), np.float32)
        out[0:D, 0:D] = a
        out[D:128, D:128] = b_
        return out

    wmats = {
        "l1_c1": W1[0].astype(bf),
        "l2_c1": W2o[0].astype(bf),
        "l1_c23": np.concatenate([W1[1], W1[2]], axis=1).astype(bf),
        "l2_c23": bd(W2[1], W2[2]).astype(bf),
        "l1_r2": bd(W1[1], W1[2]).astype(bf),
        "l2_r2": bd(W2o[1], W2[2]).astype(bf),
        "l1_r3": W1[2].astype(bf),
        "l2_r3": W2o[2].astype(bf),
        "b1_0": b1[0].reshape(D, 1),
        "b1_12": np.concatenate([b1[1], b1[2]]).reshape(128, 1),
        "b2_12": np.concatenate([b2[1], b2[2]]).reshape(128, 1),
        "b2_2": b2[2].reshape(D, 1),
        "b1_2": b1[2].reshape(D, 1),
        "bo": bo_eff.reshape(D, 1),
    }

    xpad = np.zeros((cfg["n_nodes"], 128), dtype=bf)
    xpad[:, :D] = x.astype(bf)
    xtabA = np.ascontiguousarray(np.concatenate(
        [xpad[r * npc:r * npc + splitA] for r in range(n_cores)]))
    xtabB = np.ascontiguousarray(np.concatenate(
        [xpad[r * npc + splitA:(r + 1) * npc] for r in range(n_cores)]))

    in_maps = []
    for r in range(n_cores):
        m = dict(wmats)
        m["xtabA"] = xtabA
        m["xtabB"] = xtabB
        xs = x[r * npc:(r + 1) * npc]
        m["xt"] = np.ascontiguousarray(xs.T.astype(bf))
        m["idx"] = pp["per_core"][r]["idx"]
        m["dstrel"] = pp["per_core"][r]["dstrel"]
        in_maps.append(m)
    return in_maps


_PROF_SO = "/opt/axon/libaxon_pjrt.so"


def _install_profile_shim():
    """Provide antenv.axon_hooks (absent in some containers) so
    run_bass_kernel_spmd(trace=True) can capture NTFF profiles."""
    try:
        import antenv
    except ImportError:
        return
    if getattr(antenv, "axon_hooks", None) is not None:
        return

    def _hook_factory(so_path):
        try:
            lib = ctypes.CDLL(so_path)
        except OSError:
            return None
        if not hasattr(lib, "axon_start_nrt_profile"):
            return None
        lib.axon_start_nrt_profile.argtypes = [ctypes.POINTER(ctypes.c_int64),
                                               ctypes.c_size_t]
        lib.axon_start_nrt_profile.restype = ctypes.c_int64
        lib.axon_stop_nrt_profile.argtypes = [ctypes.c_char_p]
        lib.axon_stop_nrt_profile.restype = ctypes.c_int64

        @contextlib.contextmanager
        def _hook(output_dir, device_ids):
            import jax
            jax.devices()
            if device_ids:
                ids = (ctypes.c_int64 * len(device_ids))(*device_ids)
                rc = lib.axon_start_nrt_profile(ids, len(device_ids))
            else:
                rc = lib.axon_start_nrt_profile(None, 0)
            if rc != 0:
                raise RuntimeError(f"axon_start_nrt_profile rc={rc}")
            try:
                yield
            finally:
                n = lib.axon_stop_nrt_profile(str(output_dir).encode())
                print(f"profile: {n} file(s) written to {output_dir}",
                      file=sys.stderr)

        return _hook

    mod = types.ModuleType("antenv.axon_hooks")
    _state = {"hook": _hook_factory(_PROF_SO)}
    mod.set_axon_ntff_profile_hook = lambda h: _state.__setitem__("hook", h)
    mod.get_axon_ntff_profile_hook = lambda: _state["hook"]
    sys.modules["antenv.axon_hooks"] = mod
    antenv.axon_hooks = mod
    import concourse.bass_utils as _bu
    _bu.upload_artifacts = lambda tmpdir: f"local://{tmpdir}"


_CACHE = {}


def _get_program(edge_index):
    key = hash(edge_index.tobytes())
    if key not in _CACHE:
        cfg = make_config(N_NODES, N_EDGES, N_CORES)
        new_of_old, old_of_new = relabel(cfg, edge_index)
        ei2 = np.stack([new_of_old[edge_index[0]], new_of_old[edge_index[1]]])
        pp = preprocess(cfg, ei2)
        nc = build(cfg, pp)
        _CACHE[key] = (cfg, pp, nc, new_of_old, old_of_new)
    return _CACHE[key]


def run(trace=False, **inputs):
    """Run the kernel; returns (output [N_NODES, 64] f32, exec_time_ns|None)."""
    from concourse.bass_utils import run_bass_kernel_spmd

    x = np.asarray(inputs["x"], dtype=np.float32)
    edge_index = np.asarray(inputs["edge_index"], dtype=np.int64)
    weights = {k: np.asarray(v) for k, v in inputs.items()
               if k not in ("x", "edge_index")}
    assert x.shape == (N_NODES, D) and edge_index.shape == (2, N_EDGES)

    if trace:
        _install_profile_shim()
    cfg, pp, nc, new_of_old, old_of_new = _get_program(edge_index)
    in_maps = host_inputs(cfg, pp, x[old_of_new], weights)
    res = run_bass_kernel_spmd(nc, in_maps, list(range(N_CORES)), trace=trace)
    out = np.concatenate([res.results[r]["out"] for r in range(N_CORES)],
                         axis=0).astype(np.float32)
    return out[new_of_old], res.exec_time_ns


def kernel(**inputs):
    out, _ = run(trace=False, **inputs)
    return out


# revision 32
# speedup vs baseline: 1.1115x; 1.1074x over previous
"""MixHopConv (3 GIN hop-chains, N=50000, D=64, E=800000) on 8 TRN2 NeuronCores.

Self-contained Bass kernel: kernel(**inputs) takes the full (unsharded)
inputs and returns the full [50000, 64] float32 output.

v2 strategy (nodes sharded contiguously across 8 cores, 6250 each):
  - Round-1 source tables (padded x, [n,128] bf16 rows) are host-replicated
    parameters: no startup AllGather.
  - Per round: per-edge rows are fetched with dma_gather (SWDGE, 4 queues),
    aggregated by "staircase" PE matmuls against one-hot S matrices built on
    DVE from staged dst offsets. Gather issue runs SHIFT chunks ahead of
    consumption, A-half ahead of B-half, so the 4 descriptor-generation
    queues stay busy; the Pool instruction stream contains only gathers and
    the two AllGather issues per round (placed late to avoid head-of-line
    blocking at the in-order sequencer).
  - MLP algebra is fused: chains 2+3 run stacked on 128 partitions
    (block-diagonal weights); the output projection is folded into each
    chain's second MLP layer (W2@Wo precomputed), and all projection biases
    fold into a single bo'. The output accumulator lives in SBUF.
  - Round 1 ships [a|b] shards (transposed on PE) into bounce DRAM;
    AllGather A fires mid-round, B at round end.  Round 2 ships [junk|c].
    Round 3 consumes and writes the final output.
"""
import sys
import contextlib
import ctypes
import types

import numpy as np
import ml_dtypes

for _p in ("/opt/trn_rl_repo", "/opt/pypackages"):
    if _p not in sys.path:
        sys.path.append(_p)

import concourse.bass as bass
import concourse.bass_isa as bass_isa
import concourse.mybir as mybir
import concourse.tile as tile
import concourse.bacc as bacc
from concourse.bass import AP
from concourse.masks import make_identity
from concourse.library_config import mlp as mlp_lib

N_NODES = 50000
N_EDGES = 800000
N_CORES = 8
BF16 = mybir.dt.bfloat16
F32 = mybir.dt.float32
I16 = mybir.dt.int16

D = 64
CHUNK = 512
WIN = 128

SHIFT = 3      # B-half gather lags A-half by this many chunks
AG_POS_A = 7   # AllGather-A issued after this chunk's A-gather
AG_POS_B = 1   # prev round's AllGather-B issued after this chunk's A-gather
GMAX = 1024    # max idxs per dma_gather instruction


def make_config(n_nodes, n_edges, n_cores=8):
    assert n_nodes % n_cores == 0
    npc = n_nodes // n_cores
    chunks = []  # (cbase, [(sbase, w), ...])
    off = 0
    while off < npc:
        cw = min(CHUNK, npc - off)
        slots = []
        soff = 0
        while soff < cw:
            w = min(WIN, cw - soff)
            slots.append((soff, w))
            soff += w
        chunks.append((off, slots))
        off += cw
    # region A = first chunks per core: a smaller A lets AllGather-A fire
    # earlier in the round (more wire overlap); B stays under the int16
    # table-row limit (8*(npc-splitA) <= 32767)
    ca = len(chunks) // 2 - 1
    splitA = sum(sum(w for (_, w) in sl) for (_, sl) in chunks[:ca])
    return dict(n_nodes=n_nodes, n_edges=n_edges, n_cores=n_cores, npc=npc,
                chunks=chunks, chunksA=ca, splitA=splitA)


def preprocess(cfg, edge_index):
    """Bucket/sort/pad edges; build per-core idx + dstrel arrays."""
    n_cores, npc = cfg["n_cores"], cfg["npc"]
    chunks = cfg["chunks"]
    splitA = cfg["splitA"]
    src = np.asarray(edge_index[0], dtype=np.int64)
    dst = np.asarray(edge_index[1], dtype=np.int64)

    buckets = [[[[None for _ in ch[1]] for _ in range(2)] for ch in chunks]
               for _ in range(n_cores)]
    core_of = dst // npc
    ldst = dst - core_of * npc
    cid = np.minimum(ldst // CHUNK, len(chunks) - 1)
    src_r = src // npc
    src_j = src - src_r * npc
    hid = (src_j >= splitA).astype(np.int64)
    splitB = npc - splitA
    tpos = np.where(hid == 0, src_r * splitA + src_j,
                    src_r * splitB + (src_j - splitA))
    for r in range(n_cores):
        m_r = core_of == r
        for c, (cbase, slots) in enumerate(chunks):
            m_rc = m_r & (cid == c)
            for s, (sbase, w) in enumerate(slots):
                m = m_rc & (ldst - cbase >= sbase) & (ldst - cbase < sbase + w)
                for h in range(2):
                    mh = m & (hid == h)
                    buckets[r][c][h][s] = (tpos[mh],
                                           (ldst[mh] - cbase - sbase))

    NB = []
    for c, (cbase, slots) in enumerate(chunks):
        NB_c = []
        for h in range(2):
            NB_ch = []
            for s in range(len(slots)):
                mx = max(len(buckets[r][c][h][s][0]) for r in range(n_cores))
                NB_ch.append(max(1, -(-mx // 128)))
            NB_c.append(NB_ch)
        NB.append(NB_c)
    NI = [[128 * sum(NB[c][h]) for h in range(2)] for c in range(len(chunks))]
    TOT = sum(NI[c][h] for c in range(len(chunks)) for h in range(2))
    NBTOT = sum(NB[c][h][s] for c in range(len(chunks)) for h in range(2)
                for s in range(len(chunks[c][1])))

    per_core = []
    for r in range(n_cores):
        idx_lin = np.zeros(TOT, dtype=np.int16)
        dr_lin = np.full((128, NBTOT), -1.0, dtype=np.float32)
        ioff = 0
        boff = 0
        for c in range(len(chunks)):
            for h in range(2):
                for s in range(len(chunks[c][1])):
                    srcs, drs = buckets[r][c][h][s]
                    nb = NB[c][h][s]
                    n = len(srcs)
                    idx_lin[ioff:ioff + n] = srcs.astype(np.int16)
                    for b in range(nb):
                        lo, hi = b * 128, min((b + 1) * 128, n)
                        if hi > lo:
                            dr_lin[0:hi - lo, boff + b] = drs[lo:hi]
                    ioff += nb * 128
                    boff += nb
        assert ioff == TOT and boff == NBTOT
        wrapped = idx_lin.reshape(TOT // 16, 16).T  # [16, TOT//16]
        idx_arr = np.tile(wrapped, (8, 1))
        per_core.append(dict(idx=np.ascontiguousarray(idx_arr),
                             dstrel=dr_lin.astype(ml_dtypes.bfloat16)))
    return dict(NB=NB, NI=NI, TOT=TOT, NBTOT=NBTOT, per_core=per_core)


def _bcast_mid(ap, n):
    """[P, W] AP -> [P, n, W] with middle dim broadcast."""
    return AP(ap.tensor, ap.offset, [ap.ap[0], [0, n], ap.ap[1]])


def relabel(cfg, edge_index):
    """Renumber nodes to balance in-edge load across (core, slot) buckets,
    shrinking the max-over-cores block padding. Returns (new_of_old,
    old_of_new) permutations."""
    n_nodes, n_cores, npc = cfg["n_nodes"], cfg["n_cores"], cfg["npc"]
    slotw = []
    for (cbase, slots) in cfg["chunks"]:
        slotw += [w for (_, w) in slots]
    nslot = len(slotw)
    slotw = np.asarray(slotw, np.int64)
    splitA_slots = sum(1 for i in range(nslot)
                       if sum(slotw[:i]) < cfg["splitA"])
    src = np.asarray(edge_index[0], np.int64)
    dst = np.asarray(edge_index[1], np.int64)
    deg = np.bincount(dst, minlength=n_nodes).astype(np.int64)
    nb = n_cores * nslot
    caps = np.tile(slotw, n_cores).astype(np.int64)

    # pass 1: LPT by total in-degree over all (core, slot) buckets
    load = np.zeros(nb)
    left = caps.copy()
    assign = np.empty(n_nodes, np.int32)
    for v in np.argsort(-deg, kind="stable"):
        b = int(np.argmin(np.where(left > 0, load, np.inf)))
        assign[v] = b
        load[b] += deg[v]
        left[b] -= 1
    # pass 2: regions now fixed; rebalance (A-count, B-count) within regions
    regA = (assign % nslot) < splitA_slots
    dA = np.bincount(dst[regA[src]], minlength=n_nodes).astype(np.int64)
    dB = deg - dA
    assign2 = np.empty(n_nodes, np.int32)
    for cls in (0, 1):
        inA = np.arange(nslot) < splitA_slots
        if cls == 0:
            nodes = np.where(regA)[0]
            buckets = np.where(np.tile(inA, n_cores))[0]
        else:
            nodes = np.where(~regA)[0]
            buckets = np.where(np.tile(~inA, n_cores))[0]
        bcaps = caps[buckets].copy()
        la = np.zeros(len(buckets))
        lb = np.zeros(len(buckets))
        tgtA = max(dA[nodes].sum() / len(buckets), 1.0)
        tgtB = max(dB[nodes].sum() / len(buckets), 1.0)
        for v in nodes[np.argsort(-(dA[nodes] + dB[nodes]), kind="stable")]:
            score = np.maximum((la + dA[v]) / tgtA, (lb + dB[v]) / tgtB)
            j = int(np.argmin(np.where(bcaps > 0, score, np.inf)))
            assign2[v] = buckets[j]
            la[j] += dA[v]
            lb[j] += dB[v]
            bcaps[j] -= 1

    new_of_old = np.empty(n_nodes, np.int64)
    slot_base = np.concatenate([[0], np.cumsum(slotw)])[:nslot]
    pos = np.zeros(nb, np.int64)
    for v in np.argsort(assign2, kind="stable"):
        b = assign2[v]
        core, slot = b // nslot, b % nslot
        new_of_old[v] = core * npc + slot_base[slot] + pos[b]
        pos[b] += 1
    old_of_new = np.empty(n_nodes, np.int64)
    old_of_new[new_of_old] = np.arange(n_nodes)
    return new_of_old, old_of_new


def build(cfg, pp):
    """Build the 8-core SPMD program. Returns nc."""
    n_nodes, n_cores, npc = cfg["n_nodes"], cfg["n_cores"], cfg["npc"]
    chunks = cfg["chunks"]
    ca, splitA = cfg["chunksA"], cfg["splitA"]
    splitB = npc - splitA
    nA, nB = n_cores * splitA, n_cores * splitB
    NB, NI, TOT, NBTOT = pp["NB"], pp["NI"], pp["TOT"], pp["NBTOT"]
    NCH = len(chunks)
    RG = [list(range(n_cores))]

    nc = bacc.Bacc("TRN2", target_bir_lowering=False, num_swdge_queues=4)

    # ---- parameters ----
    xtabA_in = nc.declare_dram_parameter("xtabA", [nA, 128], BF16, isOutput=False)
    xtabB_in = nc.declare_dram_parameter("xtabB", [nB, 128], BF16, isOutput=False)
    xt_in = nc.declare_dram_parameter("xt", [D, npc], BF16, isOutput=False)
    idx_in = nc.declare_dram_parameter("idx", [128, TOT // 16], I16, isOutput=False)
    dr_in = nc.declare_dram_parameter("dstrel", [128, NBTOT], BF16, isOutput=False)
    wdefs = {
        "l1_c1": ([D, D], BF16), "l2_c1": ([D, D], BF16),
        "l1_c23": ([D, 128], BF16), "l2_c23": ([128, 128], BF16),
        "l1_r2": ([128, 128], BF16), "l2_r2": ([128, 128], BF16),
        "l1_r3": ([D, D], BF16), "l2_r3": ([D, D], BF16),
        "b1_0": ([D, 1], F32), "b1_12": ([128, 1], F32),
        "b2_12": ([128, 1], F32), "b2_2": ([D, 1], F32),
        "b1_2": ([D, 1], F32), "bo": ([D, 1], F32),
    }
    w_in = {k: nc.declare_dram_parameter(k, shp, dt, isOutput=False)
            for k, (shp, dt) in wdefs.items()}
    out_ext = nc.declare_dram_parameter("out", [npc, D], F32, isOutput=True)

    # ---- internal DRAM ----
    bounceA = [nc.dram_tensor(f"bounceA{k}", [splitA, 128], BF16) for k in range(2)]
    bounceB = [nc.dram_tensor(f"bounceB{k}", [splitB, 128], BF16) for k in range(2)]
    tablesA = [nc.dram_tensor(f"tableA{k}", [nA, 128], BF16,
                              addr_space="Shared") for k in range(2)]
    tablesB = [nc.dram_tensor(f"tableB{k}", [nB, 128], BF16,
                              addr_space="Shared") for k in range(2)]

    with tile.TileContext(nc) as tc:
        nc.gpsimd.load_library(mlp_lib)
        with (
            tc.tile_pool(name="const", bufs=1) as constp,
            tc.tile_pool(name="resident", bufs=1) as resp,
            tc.tile_pool(name="gathA", bufs=SHIFT + 3) as gpoolA,
            tc.tile_pool(name="gathB", bufs=2) as gpoolB,
            tc.tile_pool(name="smat", bufs=2) as spool,
            tc.tile_pool(name="strip", bufs=2) as stripp,
            tc.tile_pool(name="psagg", bufs=2, space="PSUM") as psagg,
            tc.tile_pool(name="psmlp", bufs=2, space="PSUM") as psmlp,
            tc.tile_pool(name="pstp", bufs=1, space="PSUM") as pstp,
        ):
            # ---- constants / resident ----
            iota16 = constp.tile([128, WIN], I16)
            nc.gpsimd.iota(iota16[:], pattern=[[1, WIN]], base=0, channel_multiplier=0)
            iota_b = constp.tile([128, WIN], BF16)
            nc.vector.tensor_copy(iota_b[:], iota16[:])
            NBGMAX = max(NI[c][h] // 128 for c in range(NCH) for h in range(2))
            iota_rep = constp.tile([128, NBGMAX * WIN], BF16)
            rep = 1
            nc.vector.tensor_copy(iota_rep[:, 0:WIN], iota_b[:])
            while rep < NBGMAX:
                n = min(rep, NBGMAX - rep)
                nc.vector.tensor_copy(iota_rep[:, rep * WIN:(rep + n) * WIN],
                                      iota_rep[:, 0:n * WIN])
                rep += n
            id128 = constp.tile([128, 128], BF16)
            make_identity(nc, id128[:])
            id64 = constp.tile([D, D], F32)
            make_identity(nc, id64[:])

            # idx first: it gates the first gathers
            idx_t = resp.tile([128, TOT // 16], I16)
            nc.sync.dma_start(idx_t[:], idx_in[:])
            dr_t = resp.tile([128, NBTOT], BF16)
            nc.sync.dma_start(dr_t[:], dr_in[:])
            xt_t = resp.tile([D, npc], BF16, tag="xt", name="xt_t")
            nc.sync.dma_start(xt_t[:], xt_in[:])

            wt = {}
            for k, (shp, dt) in wdefs.items():
                wt[k] = constp.tile(shp, dt, tag=k, name=k)
                nc.sync.dma_start(wt[k][:], w_in[k][:])

            hn1 = resp.tile([128, npc], BF16, tag="hn1", name="hn1", bufs=1)
            hn2 = resp.tile([128, npc], BF16, tag="hn2", name="hn2", bufs=1)
            outacc = resp.tile([D, npc], F32, tag="oacc", name="outacc", bufs=1)
            # hn2's lower half ([junk|c] rows) is shipped but never consumed;
            # zero it once so the sim never sees uninitialized reads.
            nc.vector.memset(hn2[0:D, :], 0.0)

            # per-(chunk,half) idx offsets into idx_t / dr_t
            ioffs = {}
            boffs = {}
            io = bo_ = 0
            for c in range(NCH):
                for h in range(2):
                    ioffs[(c, h)] = io
                    boffs[(c, h)] = bo_
                    io += NI[c][h]
                    bo_ += NI[c][h] // 128
            assert io == TOT and bo_ == NBTOT

            gq = [0]

            def issue_gather(rk, c, h, tabA, tabB):
                ni = NI[c][h]
                nbg = ni // 128
                pool = gpoolA if h == 0 else gpoolB
                g = pool.tile([128, nbg, 128], BF16, tag=f"g{h}", name=f"g{h}_{rk}_{c}")
                src_ap = tabA[:] if h == 0 else tabB[:]
                io = ioffs[(c, h)]
                # split into <=GMAX-idx sub-gathers on rotating queues so each
                # fits the per-queue descriptor ring and drains in parallel
                off = 0
                while off < ni:
                    nsub = min(GMAX, ni - off)
                    b0, b1 = off // 128, (off + nsub) // 128
                    nc.gpsimd.dma_gather(
                        g[:, b0:b1, :], src_ap,
                        idx_t[:, (io + off) // 16:(io + off + nsub) // 16],
                        nsub, nsub, 128, elem_step=128, single_packet=False,
                        queue_num=gq[0] % 4)
                    gq[0] += 1
                    off += nsub
                return g

            def build_S(c, h):
                ni = NI[c][h]
                nbg = ni // 128
                slots = chunks[c][1]
                bo0 = boffs[(c, h)]
                S = spool.tile([128, nbg * WIN], BF16, tag=f"S{h}", name=f"S{h}_{c}")
                wmax = max(w for (_, w) in slots)
                if wmax == WIN:
                    nc.vector.tensor_tensor(
                        out=S[:],
                        in0=AP(dr_t[:].tensor,
                               dr_t[:, bo0:bo0 + nbg].offset,
                               [dr_t[:].ap[0], [1, nbg], [0, WIN]]),
                        in1=iota_rep[:, 0:nbg * WIN],
                        op=mybir.AluOpType.is_equal)
                else:
                    nc.vector.tensor_tensor(
                        out=AP(S[:].tensor, S[:].offset,
                               [S[:].ap[0], [WIN, nbg], [1, wmax]]),
                        in0=dr_t[:, bo0:bo0 + nbg].to_broadcast([128, nbg, wmax]),
                        in1=_bcast_mid(iota_b[:, :wmax], nbg),
                        op=mybir.AluOpType.is_equal)
                return S

            def staircase(c, gA, gB, SA, SB):
                """Accumulate this chunk's aggregation into a psum tile."""
                slots = chunks[c][1]
                ps = psagg.tile([128, CHUNK], F32, tag="agg", name=f"agg{c}")
                bcur = {0: 0, 1: 0}
                g_half = {0: gA, 1: gB}
                S_half = {0: SA, 1: SB}
                for s, (sbase, w) in enumerate(slots):
                    for h in range(2):
                        nb = NB[c][h][s]
                        for b in range(nb):
                            bi = bcur[h]
                            nc.tensor.matmul(
                                ps[:, sbase:sbase + w],
                                lhsT=g_half[h][:, bi, :],
                                rhs=S_half[h][:, bi * WIN:bi * WIN + w],
                                start=(h == 0 and b == 0),
                                stop=(h == 1 and b == nb - 1))
                            bcur[h] += 1
                return ps

            def ship(h_tile, c, bA, bB):
                """Transpose h_tile's chunk columns into bounce DRAM rows."""
                cbase, slots = chunks[c]
                cw = sum(w for (_, w) in slots)
                if cbase + cw <= splitA:
                    dstten, rowbase = bA, cbase
                else:
                    dstten, rowbase = bB, cbase - splitA
                t0 = 0
                while t0 < cw:
                    tw = min(128, cw - t0)
                    pt = pstp.tile([128, 128], BF16, tag="tp", name="tp")
                    nc.tensor.transpose(pt[:tw, :],
                                        h_tile[:, cbase + t0:cbase + t0 + tw],
                                        id128[:])
                    st = stripp.tile([128, 128], BF16, tag="shard", name="shard")
                    nc.vector.tensor_copy(st[:tw, :], pt[:tw, :])
                    nc.sync.dma_start(dstten[rowbase + t0:rowbase + t0 + tw, :],
                                      st[:tw, :])
                    t0 += tw

            def process(rk, c, gA, gB):
                cbase, slots = chunks[c]
                cw = sum(w for (_, w) in slots)
                SA = build_S(c, 0)
                SB = build_S(c, 1)
                ps = staircase(c, gA, gB, SA, SB)
                if rk == 1:
                    z1 = stripp.tile([D, CHUNK], BF16, tag="z1", name="z1")
                    nc.vector.tensor_tensor(z1[:, :cw], ps[0:D, :cw],
                                            xt_t[:, cbase:cbase + cw],
                                            op=mybir.AluOpType.add)
                    # chain 1 (hop-1): MLP0 with projection folded into layer 2
                    p1 = psmlp.tile([128, CHUNK], F32, tag="pm", name="p1")
                    nc.tensor.matmul(p1[0:D, :cw], lhsT=wt["l1_c1"][:],
                                     rhs=z1[:, :cw], start=True, stop=True)
                    m1 = stripp.tile([D, CHUNK], BF16, tag="m1", name="m1")
                    nc.scalar.activation(m1[:, :cw], p1[0:D, :cw],
                                         mybir.ActivationFunctionType.Relu,
                                         bias=wt["b1_0"][:])
                    po = psmlp.tile([128, CHUNK], F32, tag="po", name="po")
                    nc.tensor.matmul(po[0:D, :cw], lhsT=wt["l2_c1"][:],
                                     rhs=m1[:, :cw], start=True, stop=True)
                    nc.vector.tensor_copy(outacc[:, cbase:cbase + cw], po[0:D, :cw])
                    # chains 2+3 stacked: [a|b]
                    p1b = psmlp.tile([128, CHUNK], F32, tag="pm", name="p1b")
                    nc.tensor.matmul(p1b[:, :cw], lhsT=wt["l1_c23"][:],
                                     rhs=z1[:, :cw], start=True, stop=True)
                    mb = stripp.tile([128, CHUNK], BF16, tag="mb", name="mb")
                    nc.scalar.activation(mb[:, :cw], p1b[:, :cw],
                                         mybir.ActivationFunctionType.Relu,
                                         bias=wt["b1_12"][:])
                    p2b = psmlp.tile([128, CHUNK], F32, tag="po", name="p2b")
                    nc.tensor.matmul(p2b[:, :cw], lhsT=wt["l2_c23"][:],
                                     rhs=mb[:, :cw], start=True, stop=True)
                    nc.vector.tensor_tensor(hn1[:, cbase:cbase + cw],
                                            p2b[:, :cw],
                                            wt["b2_12"][:].to_broadcast([128, cw]),
                                            op=mybir.AluOpType.add)
                    ship(hn1, c, bounceA[0], bounceB[0])
                elif rk == 2:
                    zs = stripp.tile([128, CHUNK], BF16, tag="zs", name="zs")
                    nc.vector.tensor_tensor(zs[:, :cw], ps[:, :cw],
                                            hn1[:, cbase:cbase + cw],
                                            op=mybir.AluOpType.add)
                    p1 = psmlp.tile([128, CHUNK], F32, tag="pm", name="p1")
                    nc.tensor.matmul(p1[:, :cw], lhsT=wt["l1_r2"][:],
                                     rhs=zs[:, :cw], start=True, stop=True)
                    m = stripp.tile([128, CHUNK], BF16, tag="mb", name="m2")
                    nc.scalar.activation(m[:, :cw], p1[:, :cw],
                                         mybir.ActivationFunctionType.Relu,
                                         bias=wt["b1_12"][:])
                    p2 = psmlp.tile([128, CHUNK], F32, tag="po", name="p2")
                    nc.tensor.matmul(p2[:, :cw], lhsT=wt["l2_r2"][:],
                                     rhs=m[:, :cw], start=True, stop=True)
                    # p2 = [proj2 | c]
                    nc.vector.tensor_tensor(outacc[:, cbase:cbase + cw],
                                            outacc[:, cbase:cbase + cw],
                                            p2[0:D, :cw],
                                            op=mybir.AluOpType.add)
                    nc.vector.tensor_tensor(hn2[D:128, cbase:cbase + cw],
                                            p2[D:128, :cw],
                                            wt["b2_2"][:].to_broadcast([D, cw]),
                                            op=mybir.AluOpType.add)
                    ship(hn2, c, bounceA[1], bounceB[1])
                else:
                    z3 = stripp.tile([D, CHUNK], BF16, tag="z1", name="z3")
                    nc.vector.tensor_tensor(z3[:, :cw], ps[D:128, :cw],
                                            hn2[D:128, cbase:cbase + cw],
                                            op=mybir.AluOpType.add)
                    p1 = psmlp.tile([128, CHUNK], F32, tag="pm", name="p1")
                    nc.tensor.matmul(p1[0:D, :cw], lhsT=wt["l1_r3"][:],
                                     rhs=z3[:, :cw], start=True, stop=True)
                    m = stripp.tile([D, CHUNK], BF16, tag="m1", name="m3")
                    nc.scalar.activation(m[:, :cw], p1[0:D, :cw],
                                         mybir.ActivationFunctionType.Relu,
                                         bias=wt["b1_2"][:])
                    p2 = psmlp.tile([128, CHUNK], F32, tag="po", name="p2")
                    nc.tensor.matmul(p2[0:D, :cw], lhsT=wt["l2_r3"][:],
                                     rhs=m[:, :cw], start=True, stop=True)
                    fs = stripp.tile([D, CHUNK], F32, tag="fs", name="fs")
                    nc.scalar.activation(fs[:, :cw], p2[0:D, :cw],
                                         mybir.ActivationFunctionType.Identity,
                                         bias=wt["bo"][:])
                    nc.vector.tensor_tensor(fs[:, :cw], fs[:, :cw],
                                            outacc[:, cbase:cbase + cw],
                                            op=mybir.AluOpType.add)
                    t0 = 0
                    while t0 < cw:
                        tw = min(128, cw - t0)
                        pt = pstp.tile([128, D], F32, tag="ftp", name="ftp")
                        nc.tensor.transpose(pt[:tw, :], fs[:, t0:t0 + tw], id64[:])
                        os = stripp.tile([128, D], F32, tag="fout", name="fout")
                        nc.vector.tensor_copy(os[:tw, :], pt[:tw, :])
                        nc.sync.dma_start(out_ext[cbase + t0:cbase + t0 + tw, :],
                                          os[:tw, :])
                        t0 += tw

            def do_round(rk, tabA, tabB, tA_next, tB_next, bA, bB, prev_agB):
                gAs, gBs = {}, {}
                for i in range(NCH + SHIFT):
                    if i < NCH:
                        gAs[i] = issue_gather(rk, i, 0, tabA, tabB)
                    if i == AG_POS_B and prev_agB is not None:
                        # previous round's B-shard AllGather: issued here so it
                        # doesn't head-of-line-block this round's A gathers
                        pbB, ptB = prev_agB
                        nc.gpsimd.collective_compute(
                            "AllGather", mybir.AluOpType.bypass,
                            replica_groups=RG, ins=[pbB[:]], outs=[ptB[:]])
                    if i >= SHIFT:
                        c = i - SHIFT
                        gBs[c] = issue_gather(rk, c, 1, tabA, tabB)
                        process(rk, c, gAs.pop(c), gBs.pop(c))
                    # AG-A must be issued after process(ca-1)'s ship in program
                    # order (it reads those bounceA rows), hence after process.
                    if rk < 3 and i == AG_POS_A:
                        assert i - SHIFT >= ca - 1
                        nc.gpsimd.collective_compute(
                            "AllGather", mybir.AluOpType.bypass,
                            replica_groups=RG, ins=[bA[:]], outs=[tA_next[:]])

            do_round(1, xtabA_in, xtabB_in, tablesA[0], tablesB[0],
                     bounceA[0], bounceB[0], None)
            do_round(2, tablesA[0], tablesB[0], tablesA[1], tablesB[1],
                     bounceA[1], bounceB[1], (bounceB[0], tablesB[0]))
            do_round(3, tablesA[1], tablesB[1], None, None, None, None,
                     (bounceB[1], tablesB[1]))

    # Align each Pool-engine DMA's SWDGE queue with Tile's DMASW lane
    # rotation (lane = i % 8 over scheduled Pool DMA order; ucode requires a
    # lane's completion sem to be driven by a single queue).
    pool_dma_i = 0
    for f in nc.m.functions:
        for blk in f.blocks:
            for inst in blk.instructions:
                if (inst.engine == mybir.EngineType.Pool
                        and isinstance(inst, bass_isa.AnyDMAInstruction)
                        and not isinstance(inst, mybir.InstCollectiveCompute)):
                    if hasattr(inst, "queue_num"):
                        inst.queue_num = (pool_dma_i % 8) % 4
                    pool_dma_i += 1
    nc.compile()
    return nc


def host_inputs(cfg, pp, x, weights):
    """Build per-core in_maps. x: [n_nodes, 64] f32. weights: reference arrays."""
    n_cores, npc = cfg["n_cores"], cfg["npc"]
    splitA = cfg["splitA"]
    bf = ml_dtypes.bfloat16
    x = np.asarray(x, dtype=np.float32)

    W1 = [np.asarray(weights[f"W1_{i}"], np.float32) for i in range(3)]
    b1 = [np.asarray(weights[f"b1_{i}"], np.float32) for i in range(3)]
    W2 = [np.asarray(weights[f"W2_{i}"], np.float32) for i in range(3)]
    b2 = [np.asarray(weights[f"b2_{i}"], np.float32) for i in range(3)]
    Wo = np.asarray(weights["Wo"], np.float32).reshape(3, D, D)
    bo = np.asarray(weights["bo"], np.float32)

    W2o = [W2[i] @ Wo[i] for i in range(3)]
    bo_eff = bo + sum(Wo[i].T @ b2[i] for i in range(3))

    def bd(a, b_):
        out = np.zeros((128, 128), np.float32)
        out[0:D, 0:D] = a
        out[D:128, D:128] = b_
        return out

    wmats = {
        "l1_c1": W1[0].astype(bf),
        "l2_c1": W2o[0].astype(bf),
        "l1_c23": np.concatenate([W1[1], W1[2]], axis=1).astype(bf),
        "l2_c23": bd(W2[1], W2[2]).astype(bf),
        "l1_r2": bd(W1[1], W1[2]).astype(bf),
        "l2_r2": bd(W2o[1], W2[2]).astype(bf),
        "l1_r3": W1[2].astype(bf),
        "l2_r3": W2o[2].astype(bf),
        "b1_0": b1[0].reshape(D, 1),
        "b1_12": np.concatenate([b1[1], b1[2]]).reshape(128, 1),
        "b2_12": np.concatenate([b2[1], b2[2]]).reshape(128, 1),
        "b2_2": b2[2].reshape(D, 1),
        "b1_2": b1[2].reshape(D, 1),
        "bo": bo_eff.reshape(D, 1),
    }

    xpad = np.zeros((cfg["n_nodes"], 128), dtype=bf)
    xpad[:, :D] = x.astype(bf)
    xtabA = np.ascontiguousarray(np.concatenate(
        [xpad[r * npc:r * npc + splitA] for r in range(n_cores)]))
    xtabB = np.ascontiguousarray(np.concatenate(
        [xpad[r * npc + splitA:(r + 1) * npc] for r in range(n_cores)]))

    in_maps = []
    for r in range(n_cores):
        m = dict(wmats)
        m["xtabA"] = xtabA
        m["xtabB"] = xtabB
        xs = x[r * npc:(r + 1) * npc]
        m["xt"] = np.ascontiguousarray(xs.T.astype(bf))
        m["idx"] = pp["per_core"][r]["idx"]
        m["dstrel"] = pp["per_core"][r]["dstrel"]
        in_maps.append(m)
    return in_maps


_PROF_SO = "/opt/axon/libaxon_pjrt.so"


def _install_profile_shim():
    """Provide antenv.axon_hooks (absent in some containers) so
    run_bass_kernel_spmd(trace=True) can capture NTFF profiles."""
    try:
        import antenv
    except ImportError:
        return
    if getattr(antenv, "axon_hooks", None) is not None:
        return

    def _hook_factory(so_path):
        try:
            lib = ctypes.CDLL(so_path)
        except OSError:
            return None
        if not hasattr(lib, "axon_start_nrt_profile"):
            return None
        lib.axon_start_nrt_profile.argtypes = [ctypes.POINTER(ctypes.c_int64),
                                               ctypes.c_size_t]
        lib.axon_start_nrt_profile.restype = ctypes.c_int64
        lib.axon_stop_nrt_profile.argtypes = [ctypes.c_char_p]
        lib.axon_stop_nrt_profile.restype = ctypes.c_int64

        @contextlib.contextmanager
        def _hook(output_dir, device_ids):
            import jax
            jax.devices()
            if device_ids:
                ids = (ctypes.c_int64 * len(device_ids))(*device_ids)
                rc = lib.axon_start_nrt_profile(ids, len(device_ids))
            else:
                rc = lib.axon_start_nrt_profile(None, 0)
            if rc != 0:
                raise RuntimeError(f"axon_start_nrt_profile rc={rc}")
            try:
                yield
            finally:
                n = lib.axon_stop_nrt_profile(str(output_dir).encode())
                print(f"profile: {n} file(s) written to {output_dir}",
                      file=sys.stderr)

        return _hook

    mod = types.ModuleType("antenv.axon_hooks")
    _state = {"hook": _hook_factory(_PROF_SO)}
    mod.set_axon_ntff_profile_hook = lambda h: _state.__setitem__("hook", h)
    mod.get_axon_ntff_profile_hook = lambda: _state["hook"]
    sys.modules["antenv.axon_hooks"] = mod
    antenv.axon_hooks = mod
    import concourse.bass_utils as _bu
    _bu.upload_artifacts = lambda tmpdir: f"local://{tmpdir}"


_CACHE = {}


def _get_program(edge_index):
    key = hash(edge_index.tobytes())
    if key not in _CACHE:
        cfg = make_config(N_NODES, N_EDGES, N_CORES)
        new_of_old, old_of_new = relabel(cfg, edge_index)
        ei2 = np.stack([new_of_old[edge_index[0]], new_of_old[edge_index[1]]])
        pp = preprocess(cfg, ei2)
        nc = build(cfg, pp)
        _CACHE[key] = (cfg, pp, nc, new_of_old, old_of_new)
    return _CACHE[key]


def run(trace=False, **inputs):
    """Run the kernel; returns (output [N_NODES, 64] f32, exec_time_ns|None)."""
    from concourse.bass_utils import run_bass_kernel_spmd

    x = np.asarray(inputs["x"], dtype=np.float32)
    edge_index = np.asarray(inputs["edge_index"], dtype=np.int64)
    weights = {k: np.asarray(v) for k, v in inputs.items()
               if k not in ("x", "edge_index")}
    assert x.shape == (N_NODES, D) and edge_index.shape == (2, N_EDGES)

    if trace:
        _install_profile_shim()
    cfg, pp, nc, new_of_old, old_of_new = _get_program(edge_index)
    in_maps = host_inputs(cfg, pp, x[old_of_new], weights)
    res = run_bass_kernel_spmd(nc, in_maps, list(range(N_CORES)), trace=trace)
    out = np.concatenate([res.results[r]["out"] for r in range(N_CORES)],
                         axis=0).astype(np.float32)
    return out[new_of_old], res.exec_time_ns


def kernel(**inputs):
    out, _ = run(trace=False, **inputs)
    return out
